# revision 11
# baseline (speedup 1.0000x reference)
"""Trainium2 Bass kernel for a 2-layer hyperbolic GCN (HGCN) graph-pair encoder.

Sharding: data-parallel over the two graphs (cores 0-3 = graph 1, cores 4-7 =
graph 2); within a graph, 1D row-parallel sharding of layer 1 (each core owns
1024 rows of adj and computes z1/s2 for them) and 1D column-parallel sharding
of the layer-2 aggregation: each core multiplies its OWN s2 rows against its
1024-column block of adj, producing a partial full-height z2; the four
partials per graph are summed on the host. No device collective at all --
this removes the ncfw entry barrier (~30us) + AllGather (~17us) from the
critical path and lets GEMM2 start right after the chain.

Device kernel (per core, SPMD):
  warm-up PE transposes (HAM clock-gate release) while adjacency DMA streams
  GEMM1: zt1[f,i] = sum_j s1[j,f] * at1[j,i]  (bf16, fp32 PSUM, single
  accumulation sweep paced by the at1 DMA chunks)
  rowwise hyperbolic chain in fp32 as two half-streams: half 0 on DVE,
  half 1 on GpSimd; PSUM<->SBUF copies split DVE/ACT; fused scalar forms
  (artanh(min(tanh(n), 0.996)) == min(n, artanh(0.996))) kill most
  transcendentals
  HypLinear2 on r1, mobius bias add -> s2 (local rows, [node,feat] tiles)
  GEMM2 (column-parallel): zt2[f,i] partial = sum_{j local} s2[j,f]*at2[j,i]
  for ALL i, written out as bf16 partials per 1024-column chunk

All DRAM I/O uses partition-major [128, ...] layouts so DMA lines are
2-16KB contiguous per partition.

Host: input sharding/transpose/bf16 cast, D^-1/2 scaling vector, layer-1
pre-aggregation transform s1 (O(N*D^2), <0.2% of FLOPs), partial-z2 sum,
layer-2 HypAct + readout + MLP epilogue (O(N*D))."""

import math

import numpy as np
import ml_dtypes

import concourse.bass as bass
import concourse.tile as tile
from concourse import bacc, mybir
from concourse import bass_utils
from concourse.masks import make_identity

dt = mybir.dt
Alu = mybir.AluOpType
Act = mybir.ActivationFunctionType

N = 4096          # nodes per graph
F = 128           # feature dim
N_CORES = 8
CPG = 4           # cores per graph
RL = N // CPG     # rows (nodes) per core = 1024
NT = RL // 128    # node tiles per core = 8
HT = NT // 2      # node tiles per half-stream = 4
KT = N // 128     # contraction chunks = 32
IC = 4            # i-chunks for GEMM2 (1024 cols each)
LOOP_ATT = 10.0
MAXN = 1.0 - 4e-3              # Poincare ball projection radius
ATH = float(np.arctanh(MAXN))  # artanh(0.996)
MIN_NORM = 1e-15

f32 = dt.float32
bf16 = dt.bfloat16


def build_nc():
    nc = bacc.Bacc("TRN2", target_bir_lowering=False, debug=False,
                   num_devices=N_CORES)

    # partition-major external layouts
    at1_ext = nc.dram_tensor("at1", [128, KT * RL], bf16,
                             kind="ExternalInput").ap()
    at2_ext = nc.dram_tensor("at2", [128, IC * NT * 1024], bf16,
                             kind="ExternalInput").ap()
    s1_ext = nc.dram_tensor("s1", [128, KT * F], bf16,
                            kind="ExternalInput").ap()
    dinv_ext = nc.dram_tensor("dinv", [128, NT], f32, kind="ExternalInput").ap()
    w2t_ext = nc.dram_tensor("w2t", [F, F], bf16, kind="ExternalInput").ap()
    hb2_ext = nc.dram_tensor("hb2", [128, F], f32, kind="ExternalInput").ap()
    out_ext = nc.dram_tensor("out", [128, IC * 1024], bf16,
                             kind="ExternalOutput").ap()

    with tile.TileContext(nc) as tc:
        with (
            tc.tile_pool(name="big", bufs=1) as big,
            tc.tile_pool(name="work", bufs=1) as work,
            tc.tile_pool(name="ps_acc", bufs=1, space="PSUM") as ps_acc,
            tc.tile_pool(name="ps_tr", bufs=1, space="PSUM") as ps_tr,
        ):
            H = 2  # half-streams

            def halfslice(v, h):
                return v[:, h * HT * 128:(h + 1) * HT * 128]

            def tileslice(v, t):
                return v[:, t * 128:(t + 1) * 128]

            class SV:
                # one [128, NT] scalar tile; halves view columns h*HT:(h+1)*HT
                def __init__(self, name):
                    self.t = work.tile([128, NT], f32, name=name)
                def __getitem__(self, h):
                    return self.t[:, h * HT:(h + 1) * HT]
                @property
                def full(self):
                    return self.t[:]

            def sc(name):
                return SV(name)

            def wide(name, dtype=f32):
                return work.tile([128, NT * 128], dtype, name=name)

            # wide copies: half 0 on DVE, half 1 on ACT (Copy is in every
            # table set; GpSimd cannot read PSUM)
            CPE = [nc.vector.tensor_copy, nc.scalar.copy]

            def sq_norms(v, nsq, what):
                """nsq[h][:, t] = sum_f v[:, (h*HT+t)*128 + f]^2.
                Half 0: DVE fused stt+accum; half 1: ACT Square+accum
                (square is in every ACT table set -- no table swap)."""
                for t in range(HT):
                    scr = work.tile([128, 128], f32, name=f"nscr_{what}",
                                    tag="nscr0", bufs=4)
                    nc.vector.scalar_tensor_tensor(
                        scr[:], tileslice(v, t), 1.0, tileslice(v, t),
                        Alu.bypass, Alu.mult, accum_out=nsq[0][:, t:t + 1])
                for t in range(HT):
                    scr = work.tile([128, 128], f32, name=f"nscr_{what}",
                                    tag="nscr1", bufs=4)
                    g = HT + t
                    nc.scalar.activation(scr[:], tileslice(v, g), Act.Square,
                                         accum_out=nsq[1][:, t:t + 1])

            def act(outs, ins, func):
                nc.scalar.activation(outs.full, ins.full, func)

            def vmax(tiles, const):
                nc.vector.tensor_scalar_max(tiles.full, tiles.full, const)

            def recip(outs, ins):
                nc.vector.reciprocal(outs.full, ins.full)

            # ---------------- constants and small inputs ----------------
            ident = work.tile([128, 128], f32)
            make_identity(nc, ident[:])
            identb = work.tile([128, 128], bf16)
            make_identity(nc, identb[:])
            # DMA queue plan: at1 alone on the sync HWDGE queue (paces
            # GEMM1), s1 + small tensors on the scalar HWDGE queue, at2 on
            # the gpsimd SWDGE queue (needed only after the chain).
            s1_sb = big.tile([128, KT, F], bf16)
            for c in range(2):
                kc = KT // 2
                nc.scalar.dma_start(
                    s1_sb[:, c * kc:(c + 1) * kc, :],
                    s1_ext[:, c * kc * F:(c + 1) * kc * F]
                    .rearrange("p (k f) -> p k f", k=kc))
            at1_sb = big.tile([128, KT, RL], bf16)
            bounds = [round(i * KT / 14) for i in range(15)]
            for c in range(14):
                k0, k1 = bounds[c], bounds[c + 1]
                nc.sync.dma_start(
                    at1_sb[:, k0:k1, :],
                    at1_ext[:, k0 * RL:k1 * RL]
                    .rearrange("p (k i) -> p k i", k=k1 - k0))
            at2_sb = big.tile([128, IC, NT, 1024], bf16)
            for c in range(IC):
                for q in range(2):  # 1MB sub-chunks for queue granularity
                    nc.gpsimd.dma_start(
                        at2_sb[:, c, q * HT:(q + 1) * HT, :],
                        at2_ext[:, (c * NT + q * HT) * 1024:
                                (c * NT + (q + 1) * HT) * 1024]
                        .rearrange("p (t i) -> p t i", t=HT))
            dinv_sb = work.tile([128, NT], f32)
            nc.scalar.dma_start(dinv_sb[:], dinv_ext[:])
            w2t_sb = work.tile([F, F], bf16)
            nc.scalar.dma_start(w2t_sb[:], w2t_ext[:])
            hb2_sb = work.tile([128, F], f32)
            nc.scalar.dma_start(hb2_sb[:], hb2_ext[:])
            y2 = work.tile([128, 1], f32)
            y2scr = work.tile([128, F], f32)
            nc.vector.scalar_tensor_tensor(
                y2scr[:], hb2_sb[:], 1.0, hb2_sb[:],
                Alu.bypass, Alu.mult, accum_out=y2[:])

            # ---------------- PE warm-up (HAM clock-gate release) ----------
            warm_ps = ps_acc.tile([128, 1024], f32, name="acc", tag="acc",
                                  bufs=2)
            for _ in range(4):
                nc.tensor.transpose(warm_ps[:, 0:128], ident[:], ident[:])

            # ---------------- GEMM1 ----------------------------------------
            zt1_ps = ps_acc.tile([128, 1024], f32, name="acc", tag="acc",
                                 bufs=2)[:]
            for k in range(KT):
                for n in range(2):
                    nc.tensor.matmul(
                        zt1_ps[:, n * 512:(n + 1) * 512], s1_sb[:, k, :],
                        at1_sb[:, k, n * 512:(n + 1) * 512],
                        start=(k == 0), stop=(k == KT - 1))

            # PSUM [f, i]-halves -> NP z [i, f] via PE transposes
            zt1 = wide("zt1")
            z1 = wide("z1")
            for h in range(H):
                CPE[h](halfslice(zt1, h), zt1_ps[:, h * 512:(h + 1) * 512])
                tpb = ps_tr.tile([128, 512], f32, name=f"tpz{h}",
                                 tag=f"tpz{h}", bufs=1)
                for t in range(HT):
                    nc.tensor.transpose(
                        tileslice(tpb, t), tileslice(zt1, h * HT + t),
                        ident[:])
                CPE[h](halfslice(z1, h), tpb[:])

            # ------------- layer-1: r1 = relu(min(1,A/n3)*dinv*z1) ----------
            nzsq1, nz1, n3, rn3, lam1 = sc("nzsq1"), sc("nz1"), sc("n3"), sc(
                "rn3"), sc("lam1")
            sq_norms(z1, nzsq1, "z1")
            act(nz1, nzsq1, Act.Sqrt)
            nc.vector.tensor_mul(n3.full, nz1.full, dinv_sb[:])
            vmax(n3, MIN_NORM)
            recip(rn3, n3)
            nc.vector.tensor_scalar(lam1.full, rn3.full, ATH, 1.0,
                                    Alu.mult, Alu.min)
            nc.vector.tensor_mul(lam1.full, lam1.full, dinv_sb[:])
            r1 = wide("r1")
            for t in range(HT):
                nc.vector.tensor_scalar(
                    tileslice(r1, t), tileslice(z1, t),
                    lam1[0][:, t:t + 1], 0.0, Alu.mult, Alu.max)
            for t in range(HT):
                g = HT + t
                nc.scalar.activation(tileslice(r1, g), tileslice(z1, g),
                                     Act.Relu, scale=lam1[1][:, t:t + 1])
            nrsq1, nr1 = sc("nrsq1"), sc("nr1")
            sq_norms(r1, nrsq1, "r1")
            act(nr1, nrsq1, Act.Sqrt)
            vmax(nr1, MIN_NORM)

            # ---------------- layer-2 HypLinear on r1 ----------------------
            r1t = wide("r1t", bf16)
            for h in range(H):
                tpb2 = ps_tr.tile([128, 512], f32, name=f"tpz{h}",
                                  tag=f"tpz{h}", bufs=1)
                for t in range(HT):
                    nc.tensor.transpose(tileslice(tpb2, t),
                                        tileslice(r1, h * HT + t), ident[:])
                CPE[h](halfslice(r1t, h), tpb2[:])
            rxt = wide("rxt")
            for h in range(H):
                mx_ps = ps_acc.tile([128, 512], f32, name="mx_ps", tag="mx_ps",
                                    bufs=2)
                nc.tensor.matmul(mx_ps[:], w2t_sb[:], halfslice(r1t, h),
                                 start=True, stop=True)
                CPE[h](halfslice(rxt, h), mx_ps[:])
            rx = wide("rx")
            for h in range(H):
                tpb3 = ps_tr.tile([128, 512], f32, name=f"tpz{h}",
                                  tag=f"tpz{h}", bufs=1)
                for t in range(HT):
                    nc.tensor.transpose(tileslice(tpb3, t),
                                        tileslice(rxt, h * HT + t), ident[:])
                CPE[h](halfslice(rx, h), tpb3[:])

            rxnsq, rxn = sc("rxnsq"), sc("rxn")
            sq_norms(rx, rxnsq, "rx")
            act(rxn, rxnsq, Act.Sqrt)
            vmax(rxn, MIN_NORM)
            dumm = work.tile([128, 1], f32)
            # preload the Tanh table while DVE computes arg
            nc.scalar.activation(dumm[:], y2[:, 0:1], Act.Tanh)
            rnr1, clam, arg, tv = sc("rnr1"), sc("clam"), sc("arg"), sc("tv")
            recip(rnr1, nr1)
            nc.vector.tensor_scalar(clam.full, rnr1.full, ATH, 1.0,
                                    Alu.mult, Alu.min)
            nc.vector.tensor_mul(arg.full, clam.full, rxn.full)
            act(tv, arg, Act.Tanh)
            nc.vector.tensor_scalar_min(tv.full, tv.full, MAXN)
            # preload the Sqrt table while DVE computes m/xy/mobius scalars
            nc.scalar.activation(dumm[:], y2[:, 0:1], Act.Sqrt)
            # dot_rx[i] = <rx[i], hb> computed on rx directly -- runs
            # concurrently with the tanh; xy = fm * dot_rx afterwards
            dxr = sc("dxr")
            for h in range(H):
                for t in range(HT):
                    xscr = work.tile([128, 128], f32, name="xscr",
                                     tag=f"nscr{h}", bufs=4)
                    nc.vector.scalar_tensor_tensor(
                        xscr[:], tileslice(rx, h * HT + t), 1.0, hb2_sb[:],
                        Alu.bypass, Alu.mult, accum_out=dxr[h][:, t:t + 1])
            rrxn, fm = sc("rrxn"), sc("fm")
            recip(rrxn, rxn)
            nc.vector.tensor_mul(fm.full, tv.full, rrxn.full)
            xy = sc("xy")
            nc.vector.tensor_mul(xy.full, fm.full, dxr.full)
            x2, d1, den, rden = sc("x2"), sc("d1"), sc("den"), sc("rden")
            a0, alpha, b0, beta = sc("a0"), sc("alpha"), sc("b0"), sc("beta")
            nc.vector.tensor_mul(x2.full, tv.full, tv.full)
            nc.vector.tensor_scalar(d1.full, x2.full, y2[:, 0:1], None,
                                    Alu.mult)
            nc.vector.scalar_tensor_tensor(den.full, xy.full, 2.0,
                                           d1.full, Alu.mult, Alu.add)
            nc.vector.tensor_scalar(den.full, den.full, 1.0, MIN_NORM,
                                    Alu.add, Alu.max)
            nc.vector.reciprocal(rden.full, den.full)
            nc.vector.tensor_scalar(a0.full, xy.full, 2.0, 1.0,
                                    Alu.mult, Alu.add)
            nc.vector.tensor_scalar(a0.full, a0.full, y2[:, 0:1], None,
                                    Alu.add)
            nc.vector.tensor_mul(alpha.full, a0.full, rden.full)
            nc.vector.tensor_scalar(b0.full, x2.full, -1.0, 1.0,
                                    Alu.mult, Alu.add)
            nc.vector.tensor_mul(beta.full, b0.full, rden.full)
            # ||m2||^2 analytically: m2 = a*m + b*hb with ||m||=tv,
            # <m,hb>=xy, ||hb||^2=y2  =>  n2^2 = a^2 x2 + 2ab xy + b^2 y2
            n2sq, n2, mn2 = sc("n2sq"), sc("n2"), sc("mn2")
            aa, ab, bb = sc("aa"), sc("ab"), sc("bb")
            nc.vector.tensor_mul(aa.full, alpha.full, alpha.full)
            nc.vector.tensor_mul(ab.full, alpha.full, beta.full)
            nc.vector.tensor_mul(bb.full, beta.full, beta.full)
            nc.vector.tensor_mul(aa.full, aa.full, x2.full)
            nc.vector.scalar_tensor_tensor(ab.full, xy.full, 2.0, ab.full,
                                           Alu.mult, Alu.mult)
            nc.vector.tensor_scalar(bb.full, bb.full, y2[:, 0:1], None,
                                    Alu.mult)
            nc.vector.tensor_add(n2sq.full, aa.full, ab.full)
            nc.vector.tensor_add(n2sq.full, n2sq.full, bb.full)
            act(n2, n2sq, Act.Sqrt)
            vmax(n2, MIN_NORM)
            # preload the Ln table while DVE computes the artanh ratio
            nc.scalar.activation(dumm[:], y2[:, 0:1], Act.Ln)
            am1, ram, rat, ln2 = sc("am1"), sc("ram"), sc("rat"), sc("ln2")
            nc.vector.tensor_scalar_min(mn2.full, n2.full, MAXN)
            nc.vector.tensor_scalar(am1.full, mn2.full, -1.0, 1.0,
                                    Alu.mult, Alu.add)
            nc.vector.reciprocal(ram.full, am1.full)
            nc.vector.tensor_scalar(rat.full, ram.full, 2.0, -1.0,
                                    Alu.mult, Alu.add)
            act(ln2, rat, Act.Ln)
            rn2, g2, gs = sc("rn2"), sc("g2"), sc("gs")
            nc.vector.reciprocal(rn2.full, n2.full)
            nc.vector.scalar_tensor_tensor(g2.full, ln2.full, 0.5,
                                           rn2.full, Alu.mult, Alu.mult)
            nc.vector.tensor_mul(gs.full, g2.full, dinv_sb[:])
            # s2 = gs*m2 = (gs*a)*m + (gs*b)*hb  (m2 never materialized)
            ga, gb = sc("ga"), sc("gb")
            nc.vector.tensor_mul(ga.full, gs.full, alpha.full)
            nc.vector.tensor_mul(ga.full, ga.full, fm.full)   # fold fm
            nc.vector.tensor_mul(gb.full, gs.full, beta.full)
            s2b = wide("s2b", bf16)
            for g in range(NT):
                h = g // HT
                gbhb = work.tile([128, 128], f32, name="gbhb",
                                 tag=f"gbhb{h}", bufs=2)
                if h == 0:
                    nc.vector.tensor_scalar(gbhb[:], hb2_sb[:],
                                            gb.t[:, g:g + 1], None, Alu.mult)
                else:
                    nc.scalar.activation(gbhb[:], hb2_sb[:], Act.Copy,
                                         scale=gb.t[:, g:g + 1])
                nc.vector.scalar_tensor_tensor(
                    tileslice(s2b, g), tileslice(rx, g), ga.t[:, g:g + 1],
                    gbhb[:], Alu.mult, Alu.add)

            # ---------------- GEMM2 (column-parallel, partial z2) ----------
            for c in range(IC):
                zt2_ps = ps_acc.tile([128, 1024], f32, name="acc", tag="acc",
                                     bufs=2)
                for t in range(NT):
                    for n in range(2):
                        nc.tensor.matmul(
                            zt2_ps[:, n * 512:(n + 1) * 512],
                            tileslice(s2b, t),
                            at2_sb[:, c, t, n * 512:(n + 1) * 512],
                            start=(t == 0), stop=(t == NT - 1))
                zt2c = work.tile([128, 1024], bf16, name="zt2c",
                                 tag=f"zt2c{c % 2}", bufs=1)
                CPE[c % 2](zt2c[:], zt2_ps[:])  # alternates DVE / GpSimd
                nc.sync.dma_start(out_ext[:, c * 1024:(c + 1) * 1024],
                                  zt2c[:])

    nc.compile()
    return nc


# ----------------------------------------------------------------------------
# host-side math (fp32, mirrors the device numerics)
# ----------------------------------------------------------------------------

def _np_artanh(x):
    x = np.clip(x, -1.0 + 1e-6, 1.0 - 1e-6)
    return 0.5 * (np.log1p(x) - np.log1p(-x))


def _host_hb(b):
    nb = max(np.linalg.norm(b), MIN_NORM)
    hbn = min(np.tanh(nb), MAXN)
    return ((hbn / nb) * b).astype(np.float32)


def _host_s1(x, dinv, W1, b1):
    x = x.astype(np.float32)
    n = np.maximum(np.linalg.norm(x, axis=-1, keepdims=True), MIN_NORM)
    hn = np.minimum(np.tanh(n), MAXN)
    h = (hn / n) * x
    hb = _host_hb(b1)
    y2 = float(hb @ hb)
    mx = h @ W1.T
    mxn = np.maximum(np.linalg.norm(mx, axis=-1, keepdims=True), MIN_NORM)
    arg = mxn / hn * _np_artanh(hn)
    tv = np.tanh(arg)
    m = (np.minimum(tv, MAXN) / mxn) * mx
    mn = np.minimum(tv, MAXN)
    xy = (m * hb).sum(-1, keepdims=True)
    x2 = mn * mn
    den = np.maximum(1.0 + 2.0 * xy + x2 * y2, MIN_NORM)
    alpha = (1.0 + 2.0 * xy + y2) / den
    beta = (1.0 - x2) / den
    m2 = alpha * m + beta * hb
    n2 = np.maximum(np.linalg.norm(m2, axis=-1, keepdims=True), MIN_NORM)
    g = _np_artanh(np.minimum(n2, MAXN)) / n2
    return (dinv[:, None] * g) * m2


def _erf(x):
    try:
        from scipy.special import erf
        return erf(x)
    except Exception:
        return np.vectorize(math.erf)(x).astype(x.dtype)


def _pmajor(a, nt):
    """[nt*128, w] row-major -> [128, nt*w] partition-major."""
    w = a.shape[1]
    return np.ascontiguousarray(
        a.reshape(nt, 128, w).transpose(1, 0, 2).reshape(128, nt * w))


_NC_CACHE = {}


def kernel(x1, x2, adj1, adj2, dist1, dist2, W_stack, b_stack,
           W_mlp1, b_mlp1, W_mlp2, b_mlp2):
    x1 = np.asarray(x1, np.float32)
    x2 = np.asarray(x2, np.float32)
    W_stack = np.asarray(W_stack, np.float32)
    b_stack = np.asarray(b_stack, np.float32)

    if "nc" not in _NC_CACHE:
        _NC_CACHE["nc"] = build_nc()
    nc = _NC_CACHE["nc"]

    # ---- host prep per graph ----
    in_maps = [None] * N_CORES
    dinv_full = [None, None]
    for g, adj, x in ((0, adj1, x1), (1, adj2, x2)):
        a = np.asarray(adj, np.float32)[0]
        at = np.ascontiguousarray(a.T)
        idx = np.arange(N)
        at[idx, idx] += LOOP_ATT
        at_bf = at.astype(ml_dtypes.bfloat16)
        deg = at_bf.astype(np.float32).sum(axis=0)
        dinv = deg.astype(np.float32) ** -0.5
        dinv_full[g] = dinv
        s1 = _host_s1(x[0], dinv, W_stack[0], b_stack[0])
        s1_pm = _pmajor(s1.astype(ml_dtypes.bfloat16), KT)
        w2t = np.ascontiguousarray(W_stack[1].T).astype(ml_dtypes.bfloat16)
        hb2 = np.broadcast_to(_host_hb(b_stack[1]), (128, F)).copy()
        for c in range(CPG):
            core = g * CPG + c
            r0 = c * RL
            at1_pm = _pmajor(at_bf[:, r0:r0 + RL], KT)
            # at2: local j rows x all i, [128, (c,t,i)]-major
            at2_pm = np.ascontiguousarray(
                at_bf[r0:r0 + RL, :].reshape(NT, 128, IC, 1024)
                .transpose(1, 2, 0, 3).reshape(128, IC * NT * 1024))
            dinv_np = np.ascontiguousarray(
                dinv[r0:r0 + RL].reshape(NT, 128).T).astype(np.float32)
            in_maps[core] = {
                "at1": at1_pm,
                "at2": at2_pm,
                "s1": s1_pm,
                "dinv": dinv_np,
                "w2t": w2t,
                "hb2": hb2,
            }

    res = bass_utils.run_bass_kernel_spmd(nc, in_maps,
                                          core_ids=list(range(N_CORES)))

    # ---- host epilogue: partial-z2 sum + layer-2 HypAct + readout + MLP ----
    outs = []
    for g in range(2):
        zt2 = np.zeros((128, N), np.float32)
        for c in range(CPG):
            zt2 += res.results[g * CPG + c]["out"].astype(np.float32)
        z2 = zt2.T  # [N, F]
        nz = np.linalg.norm(z2, axis=-1, keepdims=True)
        n3 = np.maximum(dinv_full[g][:, None] * nz, MIN_NORM)
        lam = np.minimum(1.0, ATH / n3) * dinv_full[g][:, None]
        r2 = np.maximum(lam * z2, 0.0)
        nr2 = np.maximum(np.linalg.norm(r2, axis=-1, keepdims=True), MIN_NORM)
        h = (np.minimum(np.tanh(nr2), MAXN) / nr2) * r2
        outs.append(np.concatenate([h.mean(axis=0), h.max(axis=0)]))
    feat = np.concatenate([outs[0], outs[1], outs[0] - outs[1]]).astype(np.float32)
    zmlp = feat @ np.asarray(W_mlp1, np.float32).T + np.asarray(b_mlp1, np.float32)
    hidden = 0.5 * zmlp * (1.0 + _erf(zmlp / np.sqrt(2.0).astype(np.float32)))
    logits = hidden @ np.asarray(W_mlp2, np.float32).T + np.asarray(
        b_mlp2, np.float32)
    return logits.astype(np.float32)


# revision 41
# speedup vs baseline: 1.3838x; 1.3838x over previous
"""Trainium2 Bass kernel for a 2-layer hyperbolic GCN (HGCN) graph-pair encoder.

Sharding: data-parallel over the two graphs (cores 0-3 = graph 1, cores 4-7 =
graph 2); within a graph, 1D row-parallel sharding of layer 1 (each core owns
1024 rows of adj and computes z1/s2 for them) and 1D column-parallel sharding
of the layer-2 aggregation: each core multiplies its OWN s2 rows against its
1024-column block of adj, producing a partial full-height z2; the four
partials per graph are summed on the host. No device collective at all --
this removes the ncfw entry barrier (~30us) + AllGather (~17us) from the
critical path and lets GEMM2 start right after the chain.

Device kernel (per core, SPMD):
  warm-up PE transposes (HAM clock-gate release) while adjacency DMA streams
  GEMM1: zt1[f,i] = sum_j s1[j,f] * at1[j,i]  (bf16, fp32 PSUM, single
  accumulation sweep paced by the at1 DMA chunks)
  rowwise hyperbolic chain in fp32 as two half-streams: half 0 on DVE,
  half 1 on GpSimd; PSUM<->SBUF copies split DVE/ACT; fused scalar forms
  (artanh(min(tanh(n), 0.996)) == min(n, artanh(0.996))) kill most
  transcendentals
  HypLinear2 on r1, mobius bias add -> s2 (local rows, [node,feat] tiles)
  GEMM2 (column-parallel): zt2[f,i] partial = sum_{j local} s2[j,f]*at2[j,i]
  for ALL i, written out as bf16 partials per 1024-column chunk

All DRAM I/O uses partition-major [128, ...] layouts so DMA lines are
2-16KB contiguous per partition.

Host: input sharding/transpose/bf16 cast, D^-1/2 scaling vector, layer-1
pre-aggregation transform s1 (O(N*D^2), <0.2% of FLOPs), partial-z2 sum,
layer-2 HypAct + readout + MLP epilogue (O(N*D))."""

import math

import numpy as np
import ml_dtypes

import concourse.bass as bass
import concourse.tile as tile
from concourse import bacc, mybir
from concourse import bass_utils
from concourse.masks import make_identity

dt = mybir.dt
Alu = mybir.AluOpType
Act = mybir.ActivationFunctionType

N = 4096          # nodes per graph
F = 128           # feature dim
N_CORES = 8
CPG = 4           # cores per graph
RL = N // CPG     # rows (nodes) per core = 1024
NT = RL // 128    # node tiles per core = 8
HT = NT // 2      # node tiles per half-stream = 4
KT = N // 128     # contraction chunks = 32
IC = 4            # i-chunks for GEMM2 (1024 cols each)
LOOP_ATT = 10.0
MAXN = 1.0 - 4e-3              # Poincare ball projection radius
ATH = float(np.arctanh(MAXN))  # artanh(0.996)
MIN_NORM = 1e-15

f32 = dt.float32
bf16 = dt.bfloat16
f8 = dt.float8e4
S2 = 256.0        # fp8 scale for s2 (folded into layer-2 dinv on host)
AC = 16           # at1 DMA chunks (2 k-tiles each)
RS_EPS = 1e-30    # rsqrt guard (rsqrt(eps) saturates the min/clip forms)


def build_nc():
    nc = bacc.Bacc("TRN2", target_bir_lowering=False, debug=False,
                   num_devices=N_CORES)

    # partition-major external layouts
    # chunk-major layouts: each DMA chunk is one fully contiguous DRAM block
    at1_ext = nc.dram_tensor("at1", [AC, 128, KT * RL // AC], bf16,
                             kind="ExternalInput").ap()
    at2_ext = nc.dram_tensor("at2", [2 * IC, 128, NT * 512], f8,
                             kind="ExternalInput").ap()
    s1_ext = nc.dram_tensor("s1", [2, 128, KT * F // 2], bf16,
                            kind="ExternalInput").ap()
    # cols 0:NT = dinv/S1 (layer 1), cols NT:2NT = dinv*S2 (layer 2)
    dinv_ext = nc.dram_tensor("dinv", [128, 2 * NT], f32,
                              kind="ExternalInput").ap()
    w2t_ext = nc.dram_tensor("w2t", [F, F], bf16, kind="ExternalInput").ap()
    hb2_ext = nc.dram_tensor("hb2", [128, F], f32, kind="ExternalInput").ap()
    out_ext = nc.dram_tensor("out", [128, IC * 1024], bf16,
                             kind="ExternalOutput").ap()

    with tile.TileContext(nc) as tc:
        with (
            tc.tile_pool(name="big", bufs=1) as big,
            tc.tile_pool(name="work", bufs=1) as work,
            tc.tile_pool(name="ps_acc", bufs=1, space="PSUM") as ps_acc,
            tc.tile_pool(name="ps_tr", bufs=1, space="PSUM") as ps_tr,
        ):
            H = 2  # half-streams

            def halfslice(v, h):
                return v[:, h * HT * 128:(h + 1) * HT * 128]

            def tileslice(v, t):
                return v[:, t * 128:(t + 1) * 128]

            class SV:
                # one [128, NT] scalar tile; halves view columns h*HT:(h+1)*HT
                def __init__(self, name):
                    self.t = work.tile([128, NT], f32, name=name)
                def __getitem__(self, h):
                    return self.t[:, h * HT:(h + 1) * HT]
                @property
                def full(self):
                    return self.t[:]

            def sc(name):
                return SV(name)

            def wide(name, dtype=f32):
                return work.tile([128, NT * 128], dtype, name=name)

            # wide copies: half 0 on DVE, half 1 on ACT (Copy is in every
            # table set; GpSimd cannot read PSUM)
            CPE = [nc.vector.tensor_copy, nc.scalar.copy]

            def sq_norms(v, nsq, what):
                """nsq[h][:, t] = sum_f v[:, (h*HT+t)*128 + f]^2.
                Half 0: DVE fused stt+accum; half 1: ACT Square+accum
                (square is in every ACT table set -- no table swap)."""
                for t in range(HT):
                    scr = work.tile([128, 128], f32, name=f"nscr_{what}",
                                    tag="nscr0", bufs=4)
                    nc.vector.scalar_tensor_tensor(
                        scr[:], tileslice(v, t), 1.0, tileslice(v, t),
                        Alu.bypass, Alu.mult, accum_out=nsq[0][:, t:t + 1])
                for t in range(HT):
                    scr = work.tile([128, 128], f32, name=f"nscr_{what}",
                                    tag="nscr1", bufs=4)
                    g = HT + t
                    nc.scalar.activation(scr[:], tileslice(v, g), Act.Square,
                                         accum_out=nsq[1][:, t:t + 1])

            def act(outs, ins, func):
                nc.scalar.activation(outs.full, ins.full, func)

            def vmax(tiles, const):
                nc.vector.tensor_scalar_max(tiles.full, tiles.full, const)

            def recip(outs, ins):
                nc.vector.reciprocal(outs.full, ins.full)

            # ---------------- constants and small inputs ----------------
            ident = work.tile([128, 128], f32)
            make_identity(nc, ident[:])
            identb = work.tile([128, 128], bf16)
            make_identity(nc, identb[:])
            # DMA plan: one FIFO ring (sync/qSP) carries the big tensors in
            # dependency order -- s1, at1 (paces GEMM1), at2 (needed only by
            # GEMM2) -- so at1 never shares bandwidth; concurrent queues
            # split bandwidth by descriptor size, not priority. The small
            # tensors ride the scalar HWDGE ring so their ~2us completion
            # latencies don't serialize ahead of at1.
            at1_sb = big.tile([128, KT, RL], bf16)
            kc1 = KT // AC
            for c in range(AC):
                nc.sync.dma_start(
                    at1_sb[:, c * kc1:(c + 1) * kc1, :],
                    at1_ext[c].rearrange("p (k i) -> p k i", k=kc1))
            at2_sb = big.tile([128, IC, NT, 1024], f8)
            for c in range(IC):
                for q in range(2):
                    nc.sync.dma_start(
                        at2_sb[:, c, q * HT:(q + 1) * HT, :],
                        at2_ext[c * 2 + q].rearrange("p (t i) -> p t i", t=HT))
            s1_sb = big.tile([128, KT, F], bf16)
            for c in range(2):
                kc = KT // 2
                nc.scalar.dma_start(
                    s1_sb[:, c * kc:(c + 1) * kc, :],
                    s1_ext[c].rearrange("p (k f) -> p k f", k=kc))
            dinv_sb = work.tile([128, 2 * NT], f32)
            nc.scalar.dma_start(dinv_sb[:], dinv_ext[:])
            w2t_sb = work.tile([F, F], bf16)
            nc.scalar.dma_start(w2t_sb[:], w2t_ext[:])
            hb2_sb = work.tile([128, F], f32)
            nc.scalar.dma_start(hb2_sb[:], hb2_ext[:])
            y2 = work.tile([128, 1], f32)
            y2scr = work.tile([128, F], f32)
            nc.vector.scalar_tensor_tensor(
                y2scr[:], hb2_sb[:], 1.0, hb2_sb[:],
                Alu.bypass, Alu.mult, accum_out=y2[:])
            # preload the Sqrt table set while DMA streams (all norm
            # activations use Sqrt -- one table set until the tanh)
            dumm = work.tile([128, 1], f32)
            nc.scalar.activation(dumm[:], y2[:, 0:1], Act.Sqrt)

            # ---------------- PE warm-up (HAM clock-gate release) ----------
            warm_ps = ps_acc.tile([128, 1024], f32, name="acc", tag="acc",
                                  bufs=2)
            for _ in range(4):
                nc.tensor.transpose(warm_ps[:, 0:128], ident[:], ident[:])

            # ---------------- GEMM1 ----------------------------------------
            zt1_ps = ps_acc.tile([128, 1024], f32, name="acc", tag="acc",
                                 bufs=2)[:]
            for k in range(KT):
                for n in range(2):
                    nc.tensor.matmul(
                        zt1_ps[:, n * 512:(n + 1) * 512], s1_sb[:, k, :],
                        at1_sb[:, k, n * 512:(n + 1) * 512],
                        start=(k == 0), stop=(k == KT - 1))

            # PSUM [f, i]-halves -> NP z [i, f] via PE transposes
            zt1 = wide("zt1")
            z1 = wide("z1")
            for h in range(H):
                CPE[h](halfslice(zt1, h), zt1_ps[:, h * 512:(h + 1) * 512])
                tpb = ps_tr.tile([128, 512], f32, name=f"tpz{h}",
                                 tag=f"tpz{h}", bufs=1)
                for t in range(HT):
                    nc.tensor.transpose(
                        tileslice(tpb, t), tileslice(zt1, h * HT + t),
                        ident[:])
                CPE[h](halfslice(z1, h), tpb[:])

            # keep the PE HAM window busy through stage A (junk transposes
            # gated on z1 so they pace with the chain, not ahead of it)
            dumA = ps_acc.tile([128, 1024], f32, name="acc", tag="acc",
                               bufs=2)
            for _ in range(6):
                nc.tensor.transpose(dumA[:, 0:128], tileslice(z1, 0),
                                    ident[:])

            # ------------- layer-1: r1 = relu(min(1,A/n3)*dinv*z1) ----------
            # lam1 = min(ATH/(dinv*nz), 1)*dinv == min(ATH/nz, dinv)
            nzsq1, nz1, q1, lam1 = sc("nzsq1"), sc("nz1"), sc("q1"), sc("lam1")
            sq_norms(z1, nzsq1, "z1")
            act(nz1, nzsq1, Act.Sqrt)
            vmax(nz1, MIN_NORM)
            recip(q1, nz1)
            nc.vector.tensor_scalar(lam1.full, q1.full, ATH, None, Alu.mult)
            nc.vector.tensor_tensor(lam1.full, lam1.full, dinv_sb[:, 0:NT],
                                    Alu.min)
            r1 = wide("r1")
            for t in range(HT):
                nc.vector.tensor_scalar(
                    tileslice(r1, t), tileslice(z1, t),
                    lam1[0][:, t:t + 1], 0.0, Alu.mult, Alu.max)
            for t in range(HT):
                g = HT + t
                nc.scalar.activation(tileslice(r1, g), tileslice(z1, g),
                                     Act.Relu, scale=lam1[1][:, t:t + 1])
            nrsq1, nr1, qr1 = sc("nrsq1"), sc("nr1"), sc("qr1")
            sq_norms(r1, nrsq1, "r1")
            act(nr1, nrsq1, Act.Sqrt)
            vmax(nr1, MIN_NORM)
            recip(qr1, nr1)              # qr1 = 1/nr1

            # ---------------- layer-2 HypLinear on r1 ----------------------
            r1t = wide("r1t", bf16)
            for h in range(H):
                tpb2 = ps_tr.tile([128, 512], f32, name=f"tpz{h}",
                                  tag=f"tpz{h}", bufs=1)
                for t in range(HT):
                    nc.tensor.transpose(tileslice(tpb2, t),
                                        tileslice(r1, h * HT + t), ident[:])
                CPE[h](halfslice(r1t, h), tpb2[:])
            rxt = wide("rxt")
            for h in range(H):
                mx_ps = ps_acc.tile([128, 512], f32, name="mx_ps", tag="mx_ps",
                                    bufs=2)
                nc.tensor.matmul(mx_ps[:], w2t_sb[:], halfslice(r1t, h),
                                 start=True, stop=True)
                CPE[h](halfslice(rxt, h), mx_ps[:])
            rx = wide("rx")
            for h in range(H):
                tpb3 = ps_tr.tile([128, 512], f32, name=f"tpz{h}",
                                  tag=f"tpz{h}", bufs=1)
                for t in range(HT):
                    nc.tensor.transpose(tileslice(tpb3, t),
                                        tileslice(rxt, h * HT + t), ident[:])
                CPE[h](halfslice(rx, h), tpb3[:])

            # keep the PE warm through the mobius-scalar + s2b stretch
            dumB = ps_acc.tile([128, 1024], f32, name="acc", tag="acc",
                               bufs=2)
            for _ in range(14):
                nc.tensor.transpose(dumB[:, 0:128], tileslice(rx, 0),
                                    ident[:])

            rxnsq, rrxn, rxn = sc("rxnsq"), sc("rrxn"), sc("rxn")
            sq_norms(rx, rxnsq, "rx")
            act(rxn, rxnsq, Act.Sqrt)
            vmax(rxn, MIN_NORM)
            recip(rrxn, rxn)             # rrxn = 1/rxn
            # preload the Tanh table while DVE computes arg + dxr
            nc.scalar.activation(dumm[:], y2[:, 0:1], Act.Tanh)
            clam, arg, tv = sc("clam"), sc("arg"), sc("tv")
            nc.vector.tensor_scalar(clam.full, qr1.full, ATH, 1.0,
                                    Alu.mult, Alu.min)
            nc.vector.tensor_mul(arg.full, clam.full, rxn.full)
            act(tv, arg, Act.Tanh)
            nc.vector.tensor_scalar_min(tv.full, tv.full, MAXN)
            # preload the Sqrt table while DVE computes m/xy/mobius scalars
            nc.scalar.activation(dumm[:], y2[:, 0:1], Act.Sqrt)
            # dot_rx[i] = <rx[i], hb> computed on rx directly -- runs
            # concurrently with the tanh; xy = fm * dot_rx afterwards
            dxr = sc("dxr")
            for h in range(H):
                for t in range(HT):
                    xscr = work.tile([128, 128], f32, name="xscr",
                                     tag=f"nscr{h}", bufs=4)
                    nc.vector.scalar_tensor_tensor(
                        xscr[:], tileslice(rx, h * HT + t), 1.0, hb2_sb[:],
                        Alu.bypass, Alu.mult, accum_out=dxr[h][:, t:t + 1])
            fm = sc("fm")
            nc.vector.tensor_mul(fm.full, tv.full, rrxn.full)
            xy = sc("xy")
            nc.vector.tensor_mul(xy.full, fm.full, dxr.full)
            x2, d1, den, rden = sc("x2"), sc("d1"), sc("den"), sc("rden")
            a0, alpha, b0, beta = sc("a0"), sc("alpha"), sc("b0"), sc("beta")
            nc.vector.tensor_mul(x2.full, tv.full, tv.full)
            nc.vector.tensor_scalar(d1.full, x2.full, y2[:, 0:1], None,
                                    Alu.mult)
            nc.vector.scalar_tensor_tensor(den.full, xy.full, 2.0,
                                           d1.full, Alu.mult, Alu.add)
            nc.vector.tensor_scalar(den.full, den.full, 1.0, MIN_NORM,
                                    Alu.add, Alu.max)
            nc.vector.reciprocal(rden.full, den.full)
            nc.vector.tensor_scalar(a0.full, xy.full, 2.0, 1.0,
                                    Alu.mult, Alu.add)
            nc.vector.tensor_scalar(a0.full, a0.full, y2[:, 0:1], None,
                                    Alu.add)
            nc.vector.tensor_mul(alpha.full, a0.full, rden.full)
            nc.vector.tensor_scalar(b0.full, x2.full, -1.0, 1.0,
                                    Alu.mult, Alu.add)
            nc.vector.tensor_mul(beta.full, b0.full, rden.full)
            # ||m2||^2 analytically: m2 = a*m + b*hb with ||m||=tv,
            # <m,hb>=xy, ||hb||^2=y2  =>  n2^2 = a^2 x2 + 2ab xy + b^2 y2
            n2sq, n2, mn2 = sc("n2sq"), sc("n2"), sc("mn2")
            aa, ab, bb = sc("aa"), sc("ab"), sc("bb")
            nc.vector.tensor_mul(aa.full, alpha.full, alpha.full)
            nc.vector.tensor_mul(ab.full, alpha.full, beta.full)
            nc.vector.tensor_mul(bb.full, beta.full, beta.full)
            nc.vector.tensor_mul(aa.full, aa.full, x2.full)
            nc.vector.scalar_tensor_tensor(ab.full, xy.full, 2.0, ab.full,
                                           Alu.mult, Alu.mult)
            nc.vector.tensor_scalar(bb.full, bb.full, y2[:, 0:1], None,
                                    Alu.mult)
            nc.vector.tensor_add(n2sq.full, aa.full, ab.full)
            nc.vector.tensor_add(n2sq.full, n2sq.full, bb.full)
            rn2 = sc("rn2")
            act(n2, n2sq, Act.Sqrt)
            vmax(n2, MIN_NORM)
            # preload the Ln table while DVE computes the artanh ratio
            nc.scalar.activation(dumm[:], y2[:, 0:1], Act.Ln)
            recip(rn2, n2)
            am1, ram, rat, ln2 = sc("am1"), sc("ram"), sc("rat"), sc("ln2")
            nc.vector.tensor_scalar_min(mn2.full, n2.full, MAXN)
            nc.vector.tensor_scalar(am1.full, mn2.full, -1.0, 1.0,
                                    Alu.mult, Alu.add)
            nc.vector.reciprocal(ram.full, am1.full)
            nc.vector.tensor_scalar(rat.full, ram.full, 2.0, -1.0,
                                    Alu.mult, Alu.add)
            act(ln2, rat, Act.Ln)
            g2, gs = sc("g2"), sc("gs")
            nc.vector.scalar_tensor_tensor(g2.full, ln2.full, 0.5,
                                           rn2.full, Alu.mult, Alu.mult)
            nc.vector.tensor_mul(gs.full, g2.full, dinv_sb[:, NT:2 * NT])
            # s2 = gs*m2 = (gs*a)*m + (gs*b)*hb  (m2 never materialized)
            ga, gb = sc("ga"), sc("gb")
            nc.vector.tensor_mul(ga.full, gs.full, alpha.full)
            nc.vector.tensor_mul(ga.full, ga.full, fm.full)   # fold fm
            nc.vector.tensor_mul(gb.full, gs.full, beta.full)
            s2b = wide("s2b", f8)
            for g in range(NT):
                h = g // HT
                gbhb = work.tile([128, 128], f32, name="gbhb",
                                 tag=f"gbhb{h}", bufs=2)
                if h == 0:
                    nc.vector.tensor_scalar(gbhb[:], hb2_sb[:],
                                            gb.t[:, g:g + 1], None, Alu.mult)
                else:
                    nc.scalar.activation(gbhb[:], hb2_sb[:], Act.Copy,
                                         scale=gb.t[:, g:g + 1])
                nc.vector.scalar_tensor_tensor(
                    tileslice(s2b, g), tileslice(rx, g), ga.t[:, g:g + 1],
                    gbhb[:], Alu.mult, Alu.add)

            # ---------------- GEMM2 (column-parallel, partial z2) ----------
            for c in range(IC):
                zt2_ps = ps_acc.tile([128, 1024], f32, name="acc", tag="acc",
                                     bufs=2)
                for t in range(NT):
                    for n in range(2):
                        nc.tensor.matmul(
                            zt2_ps[:, n * 512:(n + 1) * 512],
                            tileslice(s2b, t),
                            at2_sb[:, c, t, n * 512:(n + 1) * 512],
                            start=(t == 0), stop=(t == NT - 1))
                zt2c = work.tile([128, 1024], bf16, name="zt2c",
                                 tag=f"zt2c{c % 2}", bufs=1)
                CPE[c % 2](zt2c[:], zt2_ps[:])  # alternates DVE / GpSimd
                nc.sync.dma_start(out_ext[:, c * 1024:(c + 1) * 1024],
                                  zt2c[:])

    nc.compile()
    return nc


# ----------------------------------------------------------------------------
# host-side math (fp32, mirrors the device numerics)
# ----------------------------------------------------------------------------

def _np_artanh(x):
    x = np.clip(x, -1.0 + 1e-6, 1.0 - 1e-6)
    return 0.5 * (np.log1p(x) - np.log1p(-x))


def _host_hb(b):
    nb = max(np.linalg.norm(b), MIN_NORM)
    hbn = min(np.tanh(nb), MAXN)
    return ((hbn / nb) * b).astype(np.float32)


def _host_s1(x, dinv, W1, b1):
    x = x.astype(np.float32)
    n = np.maximum(np.linalg.norm(x, axis=-1, keepdims=True), MIN_NORM)
    hn = np.minimum(np.tanh(n), MAXN)
    h = (hn / n) * x
    hb = _host_hb(b1)
    y2 = float(hb @ hb)
    mx = h @ W1.T
    mxn = np.maximum(np.linalg.norm(mx, axis=-1, keepdims=True), MIN_NORM)
    arg = mxn / hn * _np_artanh(hn)
    tv = np.tanh(arg)
    m = (np.minimum(tv, MAXN) / mxn) * mx
    mn = np.minimum(tv, MAXN)
    xy = (m * hb).sum(-1, keepdims=True)
    x2 = mn * mn
    den = np.maximum(1.0 + 2.0 * xy + x2 * y2, MIN_NORM)
    alpha = (1.0 + 2.0 * xy + y2) / den
    beta = (1.0 - x2) / den
    m2 = alpha * m + beta * hb
    n2 = np.maximum(np.linalg.norm(m2, axis=-1, keepdims=True), MIN_NORM)
    g = _np_artanh(np.minimum(n2, MAXN)) / n2
    return (dinv[:, None] * g) * m2


def _erf(x):
    try:
        from scipy.special import erf
        return erf(x)
    except Exception:
        return np.vectorize(math.erf)(x).astype(x.dtype)


def _pmajor(a, nt):
    """[nt*128, w] row-major -> [128, nt*w] partition-major."""
    w = a.shape[1]
    return np.ascontiguousarray(
        a.reshape(nt, 128, w).transpose(1, 0, 2).reshape(128, nt * w))


_NC_CACHE = {}


def kernel(x1, x2, adj1, adj2, dist1, dist2, W_stack, b_stack,
           W_mlp1, b_mlp1, W_mlp2, b_mlp2):
    x1 = np.asarray(x1, np.float32)
    x2 = np.asarray(x2, np.float32)
    W_stack = np.asarray(W_stack, np.float32)
    b_stack = np.asarray(b_stack, np.float32)

    if "nc" not in _NC_CACHE:
        _NC_CACHE["nc"] = build_nc()
    nc = _NC_CACHE["nc"]

    # ---- host prep per graph ----
    in_maps = [None] * N_CORES
    dinv_full = [None, None]
    for g, adj, x in ((0, adj1, x1), (1, adj2, x2)):
        a = np.asarray(adj, np.float32)[0]
        at = np.ascontiguousarray(a.T)
        idx = np.arange(N)
        at[idx, idx] += LOOP_ATT
        at_bf = at.astype(ml_dtypes.bfloat16)
        at_q = at.astype(ml_dtypes.float8_e4m3)
        deg = at_bf.astype(np.float32).sum(axis=0)
        dinv = deg.astype(np.float32) ** -0.5
        dinv_full[g] = dinv
        s1 = _host_s1(x[0], dinv, W_stack[0], b_stack[0])
        s1_pm = np.ascontiguousarray(
            _pmajor(s1.astype(ml_dtypes.bfloat16), KT)
            .reshape(128, 2, KT * F // 2).transpose(1, 0, 2))
        w2t = np.ascontiguousarray(W_stack[1].T).astype(ml_dtypes.bfloat16)
        hb2 = np.broadcast_to(_host_hb(b_stack[1]), (128, F)).copy()
        for c in range(CPG):
            core = g * CPG + c
            r0 = c * RL
            at1_pm = np.ascontiguousarray(
                _pmajor(at_bf[:, r0:r0 + RL], KT)
                .reshape(128, AC, KT * RL // AC).transpose(1, 0, 2))
            # at2: local j rows x all i, fp8; chunk (c,q) = i-chunk c,
            # j-tile half q, laid out [(c,q), p, (t_rel, i)]
            at2_pm = np.ascontiguousarray(
                at_q[r0:r0 + RL, :].reshape(2, HT, 128, IC, 1024)
                .transpose(3, 0, 2, 1, 4).reshape(2 * IC, 128, HT * 1024))
            dloc = dinv[r0:r0 + RL].reshape(NT, 128).T.astype(np.float32)
            dinv_np = np.ascontiguousarray(np.hstack([dloc, dloc * S2]))
            in_maps[core] = {
                "at1": at1_pm,
                "at2": at2_pm,
                "s1": s1_pm,
                "dinv": dinv_np,
                "w2t": w2t,
                "hb2": hb2,
            }

    res = bass_utils.run_bass_kernel_spmd(nc, in_maps,
                                          core_ids=list(range(N_CORES)))

    # ---- host epilogue: partial-z2 sum + layer-2 HypAct + readout + MLP ----
    outs = []
    for g in range(2):
        zt2 = np.zeros((128, N), np.float32)
        for c in range(CPG):
            zt2 += res.results[g * CPG + c]["out"].astype(np.float32)
        z2 = zt2.T / S2  # [N, F], undo the fp8 s2 scale
        nz = np.linalg.norm(z2, axis=-1, keepdims=True)
        n3 = np.maximum(dinv_full[g][:, None] * nz, MIN_NORM)
        lam = np.minimum(1.0, ATH / n3) * dinv_full[g][:, None]
        r2 = np.maximum(lam * z2, 0.0)
        nr2 = np.maximum(np.linalg.norm(r2, axis=-1, keepdims=True), MIN_NORM)
        h = (np.minimum(np.tanh(nr2), MAXN) / nr2) * r2
        outs.append(np.concatenate([h.mean(axis=0), h.max(axis=0)]))
    feat = np.concatenate([outs[0], outs[1], outs[0] - outs[1]]).astype(np.float32)
    zmlp = feat @ np.asarray(W_mlp1, np.float32).T + np.asarray(b_mlp1, np.float32)
    hidden = 0.5 * zmlp * (1.0 + _erf(zmlp / np.sqrt(2.0).astype(np.float32)))
    logits = hidden @ np.asarray(W_mlp2, np.float32).T + np.asarray(
        b_mlp2, np.float32)
    return logits.astype(np.float32)


# revision 47
# speedup vs baseline: 1.5156x; 1.0953x over previous
"""Trainium2 Bass kernel for a 2-layer hyperbolic GCN (HGCN) graph-pair encoder.

Sharding: data-parallel over the two graphs (cores 0-3 = graph 1, cores 4-7 =
graph 2); within a graph, 1D row-parallel sharding of layer 1 (each core owns
1024 rows of adj and computes z1/s2 for them) and 1D column-parallel sharding
of the layer-2 aggregation: each core multiplies its OWN s2 rows against its
1024-column block of adj, producing a partial full-height z2; the four
partials per graph are summed on the host. No device collective at all --
this removes the ncfw entry barrier (~30us) + AllGather (~17us) from the
critical path and lets GEMM2 start right after the chain.

Device kernel (per core, SPMD):
  warm-up PE transposes (HAM clock-gate release) while adjacency DMA streams
  GEMM1: zt1[f,i] = sum_j s1[j,f] * at1[j,i]  (bf16, fp32 PSUM, single
  accumulation sweep paced by the at1 DMA chunks)
  rowwise hyperbolic chain in fp32 as two half-streams: half 0 on DVE,
  half 1 on GpSimd; PSUM<->SBUF copies split DVE/ACT; fused scalar forms
  (artanh(min(tanh(n), 0.996)) == min(n, artanh(0.996))) kill most
  transcendentals
  HypLinear2 on r1, mobius bias add -> s2 (local rows, [node,feat] tiles)
  GEMM2 (column-parallel): zt2[f,i] partial = sum_{j local} s2[j,f]*at2[j,i]
  for ALL i, written out as bf16 partials per 1024-column chunk

All DRAM I/O uses partition-major [128, ...] layouts so DMA lines are
2-16KB contiguous per partition.

Host: input sharding/transpose/bf16 cast, D^-1/2 scaling vector, layer-1
pre-aggregation transform s1 (O(N*D^2), <0.2% of FLOPs), partial-z2 sum,
layer-2 HypAct + readout + MLP epilogue (O(N*D))."""

import math

import numpy as np
import ml_dtypes

import concourse.bass as bass
import concourse.tile as tile
from concourse import bacc, mybir
from concourse import bass_utils
from concourse.masks import make_identity

dt = mybir.dt
Alu = mybir.AluOpType
Act = mybir.ActivationFunctionType

N = 4096          # nodes per graph
F = 128           # feature dim
N_CORES = 8
CPG = 4           # cores per graph
RL = N // CPG     # rows (nodes) per core = 1024
NT = RL // 128    # node tiles per core = 8
HT = NT // 2      # node tiles per half-stream = 4
KT = N // 128     # contraction chunks = 32
IC = 4            # i-chunks for GEMM2 (1024 cols each)
LOOP_ATT = 10.0
MAXN = 1.0 - 4e-3              # Poincare ball projection radius
ATH = float(np.arctanh(MAXN))  # artanh(0.996)
MIN_NORM = 1e-15

f32 = dt.float32
bf16 = dt.bfloat16
f8 = dt.float8e4
S2 = 256.0        # fp8 scale for s2 (folded into layer-2 dinv on host)
AC = 16           # at1 DMA chunks (2 k-tiles each)
RS_EPS = 1e-30    # rsqrt guard (rsqrt(eps) saturates the min/clip forms)


def build_nc():
    nc = bacc.Bacc("TRN2", target_bir_lowering=False, debug=False,
                   num_devices=N_CORES)

    # partition-major external layouts
    # partition-major strided layouts (the 64KB partition stride spreads
    # DMA engine reads across HBM channels; chunk-contiguous measured ~2x
    # slower)
    at1_ext = nc.dram_tensor("at1", [128, KT * RL], bf16,
                             kind="ExternalInput").ap()
    at2_ext = nc.dram_tensor("at2", [128, IC * NT * 1024], f8,
                             kind="ExternalInput").ap()
    s1_ext = nc.dram_tensor("s1", [128, KT * F], bf16,
                            kind="ExternalInput").ap()
    # cols 0:NT = dinv/S1 (layer 1), cols NT:2NT = dinv*S2 (layer 2)
    dinv_ext = nc.dram_tensor("dinv", [128, 2 * NT], f32,
                              kind="ExternalInput").ap()
    w2t_ext = nc.dram_tensor("w2t", [F, F], bf16, kind="ExternalInput").ap()
    hb2_ext = nc.dram_tensor("hb2", [128, F], f32, kind="ExternalInput").ap()
    out_ext = nc.dram_tensor("out", [128, IC * 1024], bf16,
                             kind="ExternalOutput").ap()

    with tile.TileContext(nc) as tc:
        with (
            tc.tile_pool(name="big", bufs=1) as big,
            tc.tile_pool(name="work", bufs=1) as work,
            tc.tile_pool(name="ps_acc", bufs=1, space="PSUM") as ps_acc,
            tc.tile_pool(name="ps_tr", bufs=1, space="PSUM") as ps_tr,
        ):
            H = 2  # half-streams

            def halfslice(v, h):
                return v[:, h * HT * 128:(h + 1) * HT * 128]

            def tileslice(v, t):
                return v[:, t * 128:(t + 1) * 128]

            class SV:
                # one [128, NT] scalar tile; halves view columns h*HT:(h+1)*HT
                def __init__(self, name):
                    self.t = work.tile([128, NT], f32, name=name)
                def __getitem__(self, h):
                    return self.t[:, h * HT:(h + 1) * HT]
                @property
                def full(self):
                    return self.t[:]

            def sc(name):
                return SV(name)

            def wide(name, dtype=f32):
                return work.tile([128, NT * 128], dtype, name=name)

            # wide copies: half 0 on DVE, half 1 on ACT (Copy is in every
            # table set; GpSimd cannot read PSUM)
            CPE = [nc.vector.tensor_copy, nc.scalar.copy]

            def sq_norms(v, nsq, what):
                """nsq[h][:, t] = sum_f v[:, (h*HT+t)*128 + f]^2.
                Half 0: DVE fused stt+accum; half 1: ACT Square+accum
                (square is in every ACT table set -- no table swap)."""
                for t in range(HT):
                    scr = work.tile([128, 128], f32, name=f"nscr_{what}",
                                    tag="nscr0", bufs=4)
                    nc.vector.scalar_tensor_tensor(
                        scr[:], tileslice(v, t), 1.0, tileslice(v, t),
                        Alu.bypass, Alu.mult, accum_out=nsq[0][:, t:t + 1])
                for t in range(HT):
                    scr = work.tile([128, 128], f32, name=f"nscr_{what}",
                                    tag="nscr1", bufs=4)
                    g = HT + t
                    nc.scalar.activation(scr[:], tileslice(v, g), Act.Square,
                                         accum_out=nsq[1][:, t:t + 1])

            def act(outs, ins, func):
                nc.scalar.activation(outs.full, ins.full, func)

            def vmax(tiles, const):
                nc.vector.tensor_scalar_max(tiles.full, tiles.full, const)

            def recip(outs, ins):
                nc.vector.reciprocal(outs.full, ins.full)

            # ---------------- constants and small inputs ----------------
            ident = work.tile([128, 128], f32)
            make_identity(nc, ident[:])
            identb = work.tile([128, 128], bf16)
            make_identity(nc, identb[:])
            # DMA plan: one FIFO ring (sync/qSP) carries the big tensors in
            # dependency order -- s1, at1 (paces GEMM1), at2 (needed only by
            # GEMM2) -- so at1 never shares bandwidth; concurrent queues
            # split bandwidth by descriptor size, not priority. The small
            # tensors ride the scalar HWDGE ring so their ~2us completion
            # latencies don't serialize ahead of at1.
            at1_sb = big.tile([128, KT, RL], bf16)
            kc1 = KT // AC
            for c in range(AC):
                k0, k1 = c * kc1, (c + 1) * kc1
                nc.sync.dma_start(
                    at1_sb[:, k0:k1, :],
                    at1_ext[:, k0 * RL:k1 * RL]
                    .rearrange("p (k i) -> p k i", k=kc1))
            at2_sb = big.tile([128, IC, NT, 1024], f8)
            for c in range(IC):
                for q in range(2):
                    nc.sync.dma_start(
                        at2_sb[:, c, q * HT:(q + 1) * HT, :],
                        at2_ext[:, (c * NT + q * HT) * 1024:
                                (c * NT + (q + 1) * HT) * 1024]
                        .rearrange("p (t i) -> p t i", t=HT))
            s1_sb = big.tile([128, KT, F], bf16)
            for c in range(2):
                kc = KT // 2
                nc.scalar.dma_start(
                    s1_sb[:, c * kc:(c + 1) * kc, :],
                    s1_ext[:, c * kc * F:(c + 1) * kc * F]
                    .rearrange("p (k f) -> p k f", k=kc))
            dinv_sb = work.tile([128, 2 * NT], f32)
            nc.scalar.dma_start(dinv_sb[:], dinv_ext[:])
            w2t_sb = work.tile([F, F], bf16)
            nc.scalar.dma_start(w2t_sb[:], w2t_ext[:])
            hb2_sb = work.tile([128, F], f32)
            nc.scalar.dma_start(hb2_sb[:], hb2_ext[:])
            y2 = work.tile([128, 1], f32)
            y2scr = work.tile([128, F], f32)
            nc.vector.scalar_tensor_tensor(
                y2scr[:], hb2_sb[:], 1.0, hb2_sb[:],
                Alu.bypass, Alu.mult, accum_out=y2[:])
            # preload the Sqrt table set while DMA streams (all norm
            # activations use Sqrt -- one table set until the tanh)
            dumm = work.tile([128, 1], f32)
            nc.scalar.activation(dumm[:], y2[:, 0:1], Act.Sqrt)

            # ---------------- PE warm-up (HAM clock-gate release) ----------
            warm_ps = ps_acc.tile([128, 1024], f32, name="acc", tag="acc",
                                  bufs=2)
            for _ in range(4):
                nc.tensor.transpose(warm_ps[:, 0:128], ident[:], ident[:])

            # ---------------- GEMM1 ----------------------------------------
            zt1_ps = ps_acc.tile([128, 1024], f32, name="acc", tag="acc",
                                 bufs=2)[:]
            for k in range(KT):
                for n in range(2):
                    nc.tensor.matmul(
                        zt1_ps[:, n * 512:(n + 1) * 512], s1_sb[:, k, :],
                        at1_sb[:, k, n * 512:(n + 1) * 512],
                        start=(k == 0), stop=(k == KT - 1))

            # PSUM [f, i]-halves -> NP z [i, f] via PE transposes
            zt1 = wide("zt1")
            z1 = wide("z1")
            for h in range(H):
                CPE[h](halfslice(zt1, h), zt1_ps[:, h * 512:(h + 1) * 512])
                tpb = ps_tr.tile([128, 512], f32, name=f"tpz{h}",
                                 tag=f"tpz{h}", bufs=1)
                for t in range(HT):
                    nc.tensor.transpose(
                        tileslice(tpb, t), tileslice(zt1, h * HT + t),
                        ident[:])
                CPE[h](halfslice(z1, h), tpb[:])

            # keep the PE HAM window busy through stage A (junk transposes
            # gated on z1 so they pace with the chain, not ahead of it)
            dumA = ps_acc.tile([128, 1024], f32, name="acc", tag="acc",
                               bufs=2)
            for _ in range(6):
                nc.tensor.transpose(dumA[:, 0:128], tileslice(z1, 0),
                                    ident[:])

            # ------------- layer-1: r1 = relu(min(1,A/n3)*dinv*z1) ----------
            # lam1 = min(ATH/(dinv*nz), 1)*dinv == min(ATH/nz, dinv)
            nzsq1, nz1, q1, lam1 = sc("nzsq1"), sc("nz1"), sc("q1"), sc("lam1")
            sq_norms(z1, nzsq1, "z1")
            act(nz1, nzsq1, Act.Sqrt)
            vmax(nz1, MIN_NORM)
            recip(q1, nz1)
            nc.vector.tensor_scalar(lam1.full, q1.full, ATH, None, Alu.mult)
            nc.vector.tensor_tensor(lam1.full, lam1.full, dinv_sb[:, 0:NT],
                                    Alu.min)
            r1 = wide("r1")
            for t in range(HT):
                nc.vector.tensor_scalar(
                    tileslice(r1, t), tileslice(z1, t),
                    lam1[0][:, t:t + 1], 0.0, Alu.mult, Alu.max)
            for t in range(HT):
                g = HT + t
                nc.scalar.activation(tileslice(r1, g), tileslice(z1, g),
                                     Act.Relu, scale=lam1[1][:, t:t + 1])
            nrsq1, nr1, qr1 = sc("nrsq1"), sc("nr1"), sc("qr1")
            sq_norms(r1, nrsq1, "r1")
            act(nr1, nrsq1, Act.Sqrt)
            vmax(nr1, MIN_NORM)
            recip(qr1, nr1)              # qr1 = 1/nr1

            # ---------------- layer-2 HypLinear on r1 ----------------------
            r1t = wide("r1t", bf16)
            for h in range(H):
                tpb2 = ps_tr.tile([128, 512], f32, name=f"tpz{h}",
                                  tag=f"tpz{h}", bufs=1)
                for t in range(HT):
                    nc.tensor.transpose(tileslice(tpb2, t),
                                        tileslice(r1, h * HT + t), ident[:])
                CPE[h](halfslice(r1t, h), tpb2[:])
            rxt = wide("rxt")
            for h in range(H):
                mx_ps = ps_acc.tile([128, 512], f32, name="mx_ps", tag="mx_ps",
                                    bufs=2)
                nc.tensor.matmul(mx_ps[:], w2t_sb[:], halfslice(r1t, h),
                                 start=True, stop=True)
                CPE[h](halfslice(rxt, h), mx_ps[:])
            rx = wide("rx")
            for h in range(H):
                tpb3 = ps_tr.tile([128, 512], f32, name=f"tpz{h}",
                                  tag=f"tpz{h}", bufs=1)
                for t in range(HT):
                    nc.tensor.transpose(tileslice(tpb3, t),
                                        tileslice(rxt, h * HT + t), ident[:])
                CPE[h](halfslice(rx, h), tpb3[:])

            # keep the PE warm through the mobius-scalar + s2b stretch
            dumB = ps_acc.tile([128, 1024], f32, name="acc", tag="acc",
                               bufs=2)
            for _ in range(14):
                nc.tensor.transpose(dumB[:, 0:128], tileslice(rx, 0),
                                    ident[:])

            rxnsq, rrxn, rxn = sc("rxnsq"), sc("rrxn"), sc("rxn")
            sq_norms(rx, rxnsq, "rx")
            act(rxn, rxnsq, Act.Sqrt)
            vmax(rxn, MIN_NORM)
            recip(rrxn, rxn)             # rrxn = 1/rxn
            # preload the Tanh table while DVE computes arg + dxr
            nc.scalar.activation(dumm[:], y2[:, 0:1], Act.Tanh)
            clam, arg, tv = sc("clam"), sc("arg"), sc("tv")
            nc.vector.tensor_scalar(clam.full, qr1.full, ATH, 1.0,
                                    Alu.mult, Alu.min)
            nc.vector.tensor_mul(arg.full, clam.full, rxn.full)
            act(tv, arg, Act.Tanh)
            nc.vector.tensor_scalar_min(tv.full, tv.full, MAXN)
            # preload the Sqrt table while DVE computes m/xy/mobius scalars
            nc.scalar.activation(dumm[:], y2[:, 0:1], Act.Sqrt)
            # dot_rx[i] = <rx[i], hb> computed on rx directly -- runs
            # concurrently with the tanh; xy = fm * dot_rx afterwards
            dxr = sc("dxr")
            for h in range(H):
                for t in range(HT):
                    xscr = work.tile([128, 128], f32, name="xscr",
                                     tag=f"nscr{h}", bufs=4)
                    nc.vector.scalar_tensor_tensor(
                        xscr[:], tileslice(rx, h * HT + t), 1.0, hb2_sb[:],
                        Alu.bypass, Alu.mult, accum_out=dxr[h][:, t:t + 1])
            fm = sc("fm")
            nc.vector.tensor_mul(fm.full, tv.full, rrxn.full)
            xy = sc("xy")
            nc.vector.tensor_mul(xy.full, fm.full, dxr.full)
            x2, d1, den, rden = sc("x2"), sc("d1"), sc("den"), sc("rden")
            a0, alpha, b0, beta = sc("a0"), sc("alpha"), sc("b0"), sc("beta")
            nc.vector.tensor_mul(x2.full, tv.full, tv.full)
            nc.vector.tensor_scalar(d1.full, x2.full, y2[:, 0:1], None,
                                    Alu.mult)
            nc.vector.scalar_tensor_tensor(den.full, xy.full, 2.0,
                                           d1.full, Alu.mult, Alu.add)
            nc.vector.tensor_scalar(den.full, den.full, 1.0, MIN_NORM,
                                    Alu.add, Alu.max)
            nc.vector.reciprocal(rden.full, den.full)
            nc.vector.tensor_scalar(a0.full, xy.full, 2.0, 1.0,
                                    Alu.mult, Alu.add)
            nc.vector.tensor_scalar(a0.full, a0.full, y2[:, 0:1], None,
                                    Alu.add)
            nc.vector.tensor_mul(alpha.full, a0.full, rden.full)
            nc.vector.tensor_scalar(b0.full, x2.full, -1.0, 1.0,
                                    Alu.mult, Alu.add)
            nc.vector.tensor_mul(beta.full, b0.full, rden.full)
            # ||m2||^2 analytically: m2 = a*m + b*hb with ||m||=tv,
            # <m,hb>=xy, ||hb||^2=y2  =>  n2^2 = a^2 x2 + 2ab xy + b^2 y2
            n2sq, n2, mn2 = sc("n2sq"), sc("n2"), sc("mn2")
            aa, ab, bb = sc("aa"), sc("ab"), sc("bb")
            nc.vector.tensor_mul(aa.full, alpha.full, alpha.full)
            nc.vector.tensor_mul(ab.full, alpha.full, beta.full)
            nc.vector.tensor_mul(bb.full, beta.full, beta.full)
            nc.vector.tensor_mul(aa.full, aa.full, x2.full)
            nc.vector.scalar_tensor_tensor(ab.full, xy.full, 2.0, ab.full,
                                           Alu.mult, Alu.mult)
            nc.vector.tensor_scalar(bb.full, bb.full, y2[:, 0:1], None,
                                    Alu.mult)
            nc.vector.tensor_add(n2sq.full, aa.full, ab.full)
            nc.vector.tensor_add(n2sq.full, n2sq.full, bb.full)
            rn2 = sc("rn2")
            act(n2, n2sq, Act.Sqrt)
            vmax(n2, MIN_NORM)
            # preload the Ln table while DVE computes the artanh ratio
            nc.scalar.activation(dumm[:], y2[:, 0:1], Act.Ln)
            recip(rn2, n2)
            am1, ram, rat, ln2 = sc("am1"), sc("ram"), sc("rat"), sc("ln2")
            nc.vector.tensor_scalar_min(mn2.full, n2.full, MAXN)
            nc.vector.tensor_scalar(am1.full, mn2.full, -1.0, 1.0,
                                    Alu.mult, Alu.add)
            nc.vector.reciprocal(ram.full, am1.full)
            nc.vector.tensor_scalar(rat.full, ram.full, 2.0, -1.0,
                                    Alu.mult, Alu.add)
            act(ln2, rat, Act.Ln)
            g2, gs = sc("g2"), sc("gs")
            nc.vector.scalar_tensor_tensor(g2.full, ln2.full, 0.5,
                                           rn2.full, Alu.mult, Alu.mult)
            nc.vector.tensor_mul(gs.full, g2.full, dinv_sb[:, NT:2 * NT])
            # s2 = gs*m2 = (gs*a)*m + (gs*b)*hb  (m2 never materialized)
            ga, gb = sc("ga"), sc("gb")
            nc.vector.tensor_mul(ga.full, gs.full, alpha.full)
            nc.vector.tensor_mul(ga.full, ga.full, fm.full)   # fold fm
            nc.vector.tensor_mul(gb.full, gs.full, beta.full)
            s2b = wide("s2b", f8)
            for g in range(NT):
                h = g // HT
                gbhb = work.tile([128, 128], f32, name="gbhb",
                                 tag=f"gbhb{h}", bufs=2)
                if h == 0:
                    nc.vector.tensor_scalar(gbhb[:], hb2_sb[:],
                                            gb.t[:, g:g + 1], None, Alu.mult)
                else:
                    nc.scalar.activation(gbhb[:], hb2_sb[:], Act.Copy,
                                         scale=gb.t[:, g:g + 1])
                nc.vector.scalar_tensor_tensor(
                    tileslice(s2b, g), tileslice(rx, g), ga.t[:, g:g + 1],
                    gbhb[:], Alu.mult, Alu.add)

            # ---------------- GEMM2 (column-parallel, partial z2) ----------
            # fp8 DoubleRow: each matmul contracts a PAIR of j-tiles
            # (lhsT [128,2,128] = two adjacent s2b tiles, rhs [128,2,512])
            s2b3 = s2b[:].rearrange("p (t f) -> p t f", t=NT)
            for c in range(IC):
                zt2_ps = ps_acc.tile([128, 1024], f32, name="acc", tag="acc",
                                     bufs=2)
                for tt in range(NT // 2):
                    for n in range(2):
                        nc.tensor.matmul(
                            zt2_ps[:, n * 512:(n + 1) * 512],
                            s2b3[:, 2 * tt:2 * tt + 2, :],
                            at2_sb[:, c, 2 * tt:2 * tt + 2,
                                   n * 512:(n + 1) * 512],
                            start=(tt == 0), stop=(tt == NT // 2 - 1),
                            perf_mode=mybir.MatmulPerfMode.DoubleRow)
                zt2c = work.tile([128, 1024], bf16, name="zt2c",
                                 tag=f"zt2c{c % 2}", bufs=1)
                CPE[c % 2](zt2c[:], zt2_ps[:])  # alternates DVE / GpSimd
                nc.sync.dma_start(out_ext[:, c * 1024:(c + 1) * 1024],
                                  zt2c[:])

    nc.compile()
    return nc


# ----------------------------------------------------------------------------
# host-side math (fp32, mirrors the device numerics)
# ----------------------------------------------------------------------------

def _np_artanh(x):
    x = np.clip(x, -1.0 + 1e-6, 1.0 - 1e-6)
    return 0.5 * (np.log1p(x) - np.log1p(-x))


def _host_hb(b):
    nb = max(np.linalg.norm(b), MIN_NORM)
    hbn = min(np.tanh(nb), MAXN)
    return ((hbn / nb) * b).astype(np.float32)


def _host_s1(x, dinv, W1, b1):
    x = x.astype(np.float32)
    n = np.maximum(np.linalg.norm(x, axis=-1, keepdims=True), MIN_NORM)
    hn = np.minimum(np.tanh(n), MAXN)
    h = (hn / n) * x
    hb = _host_hb(b1)
    y2 = float(hb @ hb)
    mx = h @ W1.T
    mxn = np.maximum(np.linalg.norm(mx, axis=-1, keepdims=True), MIN_NORM)
    arg = mxn / hn * _np_artanh(hn)
    tv = np.tanh(arg)
    m = (np.minimum(tv, MAXN) / mxn) * mx
    mn = np.minimum(tv, MAXN)
    xy = (m * hb).sum(-1, keepdims=True)
    x2 = mn * mn
    den = np.maximum(1.0 + 2.0 * xy + x2 * y2, MIN_NORM)
    alpha = (1.0 + 2.0 * xy + y2) / den
    beta = (1.0 - x2) / den
    m2 = alpha * m + beta * hb
    n2 = np.maximum(np.linalg.norm(m2, axis=-1, keepdims=True), MIN_NORM)
    g = _np_artanh(np.minimum(n2, MAXN)) / n2
    return (dinv[:, None] * g) * m2


def _erf(x):
    try:
        from scipy.special import erf
        return erf(x)
    except Exception:
        return np.vectorize(math.erf)(x).astype(x.dtype)


def _pmajor(a, nt):
    """[nt*128, w] row-major -> [128, nt*w] partition-major."""
    w = a.shape[1]
    return np.ascontiguousarray(
        a.reshape(nt, 128, w).transpose(1, 0, 2).reshape(128, nt * w))


_NC_CACHE = {}


def kernel(x1, x2, adj1, adj2, dist1, dist2, W_stack, b_stack,
           W_mlp1, b_mlp1, W_mlp2, b_mlp2):
    x1 = np.asarray(x1, np.float32)
    x2 = np.asarray(x2, np.float32)
    W_stack = np.asarray(W_stack, np.float32)
    b_stack = np.asarray(b_stack, np.float32)

    if "nc" not in _NC_CACHE:
        _NC_CACHE["nc"] = build_nc()
    nc = _NC_CACHE["nc"]

    # ---- host prep per graph ----
    in_maps = [None] * N_CORES
    dinv_full = [None, None]
    for g, adj, x in ((0, adj1, x1), (1, adj2, x2)):
        a = np.asarray(adj, np.float32)[0]
        at = np.ascontiguousarray(a.T)
        idx = np.arange(N)
        at[idx, idx] += LOOP_ATT
        at_bf = at.astype(ml_dtypes.bfloat16)
        at_q = at.astype(ml_dtypes.float8_e4m3)
        deg = at_bf.astype(np.float32).sum(axis=0)
        dinv = deg.astype(np.float32) ** -0.5
        dinv_full[g] = dinv
        s1 = _host_s1(x[0], dinv, W_stack[0], b_stack[0])
        s1_pm = _pmajor(s1.astype(ml_dtypes.bfloat16), KT)
        w2t = np.ascontiguousarray(W_stack[1].T).astype(ml_dtypes.bfloat16)
        hb2 = np.broadcast_to(_host_hb(b_stack[1]), (128, F)).copy()
        for c in range(CPG):
            core = g * CPG + c
            r0 = c * RL
            at1_pm = _pmajor(at_bf[:, r0:r0 + RL], KT)
            # at2: local j rows x all i, fp8, [128, (c,t,i)]-major
            at2_pm = np.ascontiguousarray(
                at_q[r0:r0 + RL, :].reshape(NT, 128, IC, 1024)
                .transpose(1, 2, 0, 3).reshape(128, IC * NT * 1024))
            dloc = dinv[r0:r0 + RL].reshape(NT, 128).T.astype(np.float32)
            dinv_np = np.ascontiguousarray(np.hstack([dloc, dloc * S2]))
            in_maps[core] = {
                "at1": at1_pm,
                "at2": at2_pm,
                "s1": s1_pm,
                "dinv": dinv_np,
                "w2t": w2t,
                "hb2": hb2,
            }

    res = bass_utils.run_bass_kernel_spmd(nc, in_maps,
                                          core_ids=list(range(N_CORES)))

    # ---- host epilogue: partial-z2 sum + layer-2 HypAct + readout + MLP ----
    outs = []
    for g in range(2):
        zt2 = np.zeros((128, N), np.float32)
        for c in range(CPG):
            zt2 += res.results[g * CPG + c]["out"].astype(np.float32)
        z2 = zt2.T / S2  # [N, F], undo the fp8 s2 scale
        nz = np.linalg.norm(z2, axis=-1, keepdims=True)
        n3 = np.maximum(dinv_full[g][:, None] * nz, MIN_NORM)
        lam = np.minimum(1.0, ATH / n3) * dinv_full[g][:, None]
        r2 = np.maximum(lam * z2, 0.0)
        nr2 = np.maximum(np.linalg.norm(r2, axis=-1, keepdims=True), MIN_NORM)
        h = (np.minimum(np.tanh(nr2), MAXN) / nr2) * r2
        outs.append(np.concatenate([h.mean(axis=0), h.max(axis=0)]))
    feat = np.concatenate([outs[0], outs[1], outs[0] - outs[1]]).astype(np.float32)
    zmlp = feat @ np.asarray(W_mlp1, np.float32).T + np.asarray(b_mlp1, np.float32)
    hidden = 0.5 * zmlp * (1.0 + _erf(zmlp / np.sqrt(2.0).astype(np.float32)))
    logits = hidden @ np.asarray(W_mlp2, np.float32).T + np.asarray(
        b_mlp2, np.float32)
    return logits.astype(np.float32)


# revision 49
# speedup vs baseline: 1.5289x; 1.0088x over previous
"""Trainium2 Bass kernel for a 2-layer hyperbolic GCN (HGCN) graph-pair encoder.

Sharding: data-parallel over the two graphs (cores 0-3 = graph 1, cores 4-7 =
graph 2); within a graph, 1D row-parallel sharding of layer 1 (each core owns
1024 rows of adj and computes z1/s2 for them) and 1D column-parallel sharding
of the layer-2 aggregation: each core multiplies its OWN s2 rows against its
1024-column block of adj, producing a partial full-height z2; the four
partials per graph are summed on the host. No device collective at all --
this removes the ncfw entry barrier (~30us) + AllGather (~17us) from the
critical path and lets GEMM2 start right after the chain.

Device kernel (per core, SPMD):
  warm-up PE transposes (HAM clock-gate release) while adjacency DMA streams
  GEMM1: zt1[f,i] = sum_j s1[j,f] * at1[j,i]  (bf16, fp32 PSUM, single
  accumulation sweep paced by the at1 DMA chunks)
  rowwise hyperbolic chain in fp32 as two half-streams: half 0 on DVE,
  half 1 on GpSimd; PSUM<->SBUF copies split DVE/ACT; fused scalar forms
  (artanh(min(tanh(n), 0.996)) == min(n, artanh(0.996))) kill most
  transcendentals
  HypLinear2 on r1, mobius bias add -> s2 (local rows, [node,feat] tiles)
  GEMM2 (column-parallel): zt2[f,i] partial = sum_{j local} s2[j,f]*at2[j,i]
  for ALL i, written out as bf16 partials per 1024-column chunk

All DRAM I/O uses partition-major [128, ...] layouts so DMA lines are
2-16KB contiguous per partition.

Host: input sharding/transpose/bf16 cast, D^-1/2 scaling vector, layer-1
pre-aggregation transform s1 (O(N*D^2), <0.2% of FLOPs), partial-z2 sum,
layer-2 HypAct + readout + MLP epilogue (O(N*D))."""

import math

import numpy as np
import ml_dtypes

import concourse.bass as bass
import concourse.tile as tile
from concourse import bacc, mybir
from concourse import bass_utils
from concourse.masks import make_identity

dt = mybir.dt
Alu = mybir.AluOpType
Act = mybir.ActivationFunctionType

N = 4096          # nodes per graph
F = 128           # feature dim
N_CORES = 8
CPG = 4           # cores per graph
RL = N // CPG     # rows (nodes) per core = 1024
NT = RL // 128    # node tiles per core = 8
HT = NT // 2      # node tiles per half-stream = 4
KT = N // 128     # contraction chunks = 32
IC = 4            # i-chunks for GEMM2 (1024 cols each)
LOOP_ATT = 10.0
MAXN = 1.0 - 4e-3              # Poincare ball projection radius
ATH = float(np.arctanh(MAXN))  # artanh(0.996)
MIN_NORM = 1e-15

f32 = dt.float32
bf16 = dt.bfloat16
f8 = dt.float8e4
S2 = 256.0        # fp8 scale for s2 (folded into layer-2 dinv on host)
AC = 16           # at1 DMA chunks (2 k-tiles each)
RS_EPS = 1e-30    # rsqrt guard (rsqrt(eps) saturates the min/clip forms)


def build_nc():
    nc = bacc.Bacc("TRN2", target_bir_lowering=False, debug=False,
                   num_devices=N_CORES)

    # partition-major external layouts
    # partition-major strided layouts (the 64KB partition stride spreads
    # DMA engine reads across HBM channels; chunk-contiguous measured ~2x
    # slower)
    at1_ext = nc.dram_tensor("at1", [128, KT * RL], bf16,
                             kind="ExternalInput").ap()
    at2_ext = nc.dram_tensor("at2", [128, IC * NT * 1024], f8,
                             kind="ExternalInput").ap()
    s1_ext = nc.dram_tensor("s1", [128, KT * F], bf16,
                            kind="ExternalInput").ap()
    # cols 0:NT = dinv/S1 (layer 1), cols NT:2NT = dinv*S2 (layer 2)
    dinv_ext = nc.dram_tensor("dinv", [128, 2 * NT], f32,
                              kind="ExternalInput").ap()
    w2t_ext = nc.dram_tensor("w2t", [F, F], bf16, kind="ExternalInput").ap()
    hb2_ext = nc.dram_tensor("hb2", [128, F], f32, kind="ExternalInput").ap()
    out_ext = nc.dram_tensor("out", [128, IC * 1024], bf16,
                             kind="ExternalOutput").ap()

    with tile.TileContext(nc) as tc:
        with (
            tc.tile_pool(name="big", bufs=1) as big,
            tc.tile_pool(name="work", bufs=1) as work,
            tc.tile_pool(name="ps_acc", bufs=1, space="PSUM") as ps_acc,
            tc.tile_pool(name="ps_tr", bufs=1, space="PSUM") as ps_tr,
        ):
            H = 2  # half-streams

            def halfslice(v, h):
                return v[:, h * HT * 128:(h + 1) * HT * 128]

            def tileslice(v, t):
                return v[:, t * 128:(t + 1) * 128]

            class SV:
                # one [128, NT] scalar tile; halves view columns h*HT:(h+1)*HT
                def __init__(self, name):
                    self.t = work.tile([128, NT], f32, name=name)
                def __getitem__(self, h):
                    return self.t[:, h * HT:(h + 1) * HT]
                @property
                def full(self):
                    return self.t[:]

            def sc(name):
                return SV(name)

            def wide(name, dtype=f32):
                return work.tile([128, NT * 128], dtype, name=name)

            # wide copies: half 0 on DVE, half 1 on ACT (Copy is in every
            # table set; GpSimd cannot read PSUM)
            CPE = [nc.vector.tensor_copy, nc.scalar.copy]

            NDV = 6  # norm tiles on DVE; rest on ACT (ACT accum-read is 3.4x
            #          slower, so the split is 6/2, not 4/4)

            def sq_norms(v, nsq, what):
                """nsq.t[:, t] = sum_f v[:, t*128 + f]^2. First NDV tiles:
                DVE fused stt+accum; rest: ACT Square+accum (square is in
                every ACT table set -- no table swap)."""
                for t in range(NDV):
                    scr = work.tile([128, 128], f32, name=f"nscr_{what}",
                                    tag="nscr0", bufs=4)
                    nc.vector.scalar_tensor_tensor(
                        scr[:], tileslice(v, t), 1.0, tileslice(v, t),
                        Alu.bypass, Alu.mult, accum_out=nsq.t[:, t:t + 1])
                for t in range(NDV, NT):
                    scr = work.tile([128, 128], f32, name=f"nscr_{what}",
                                    tag="nscr1", bufs=4)
                    nc.scalar.activation(scr[:], tileslice(v, t), Act.Square,
                                         accum_out=nsq.t[:, t:t + 1])

            def act(outs, ins, func):
                nc.scalar.activation(outs.full, ins.full, func)

            def vmax(tiles, const):
                nc.vector.tensor_scalar_max(tiles.full, tiles.full, const)

            def recip(outs, ins):
                nc.vector.reciprocal(outs.full, ins.full)

            # ---------------- constants and small inputs ----------------
            ident = work.tile([128, 128], f32)
            make_identity(nc, ident[:])
            identb = work.tile([128, 128], bf16)
            make_identity(nc, identb[:])
            # DMA plan: one FIFO ring (sync/qSP) carries the big tensors in
            # dependency order -- s1, at1 (paces GEMM1), at2 (needed only by
            # GEMM2) -- so at1 never shares bandwidth; concurrent queues
            # split bandwidth by descriptor size, not priority. The small
            # tensors ride the scalar HWDGE ring so their ~2us completion
            # latencies don't serialize ahead of at1.
            at1_sb = big.tile([128, KT, RL], bf16)
            kc1 = KT // AC
            for c in range(AC):
                k0, k1 = c * kc1, (c + 1) * kc1
                nc.sync.dma_start(
                    at1_sb[:, k0:k1, :],
                    at1_ext[:, k0 * RL:k1 * RL]
                    .rearrange("p (k i) -> p k i", k=kc1))
            at2_sb = big.tile([128, IC, NT, 1024], f8)
            for c in range(IC):
                for q in range(2):
                    nc.sync.dma_start(
                        at2_sb[:, c, q * HT:(q + 1) * HT, :],
                        at2_ext[:, (c * NT + q * HT) * 1024:
                                (c * NT + (q + 1) * HT) * 1024]
                        .rearrange("p (t i) -> p t i", t=HT))
            s1_sb = big.tile([128, KT, F], bf16)
            for c in range(2):
                kc = KT // 2
                nc.scalar.dma_start(
                    s1_sb[:, c * kc:(c + 1) * kc, :],
                    s1_ext[:, c * kc * F:(c + 1) * kc * F]
                    .rearrange("p (k f) -> p k f", k=kc))
            dinv_sb = work.tile([128, 2 * NT], f32)
            nc.scalar.dma_start(dinv_sb[:], dinv_ext[:])
            w2t_sb = work.tile([F, F], bf16)
            nc.scalar.dma_start(w2t_sb[:], w2t_ext[:])
            hb2_sb = work.tile([128, F], f32)
            nc.scalar.dma_start(hb2_sb[:], hb2_ext[:])
            y2 = work.tile([128, 1], f32)
            y2scr = work.tile([128, F], f32)
            nc.vector.scalar_tensor_tensor(
                y2scr[:], hb2_sb[:], 1.0, hb2_sb[:],
                Alu.bypass, Alu.mult, accum_out=y2[:])
            # preload the Sqrt table set while DMA streams (all norm
            # activations use Sqrt -- one table set until the tanh)
            dumm = work.tile([128, 1], f32)
            nc.scalar.activation(dumm[:], y2[:, 0:1], Act.Sqrt)

            # ---------------- PE warm-up (HAM clock-gate release) ----------
            warm_ps = ps_acc.tile([128, 1024], f32, name="acc", tag="acc",
                                  bufs=2)
            for _ in range(4):
                nc.tensor.transpose(warm_ps[:, 0:128], ident[:], ident[:])

            # ---------------- GEMM1 ----------------------------------------
            zt1_ps = ps_acc.tile([128, 1024], f32, name="acc", tag="acc",
                                 bufs=2)[:]
            for k in range(KT):
                for n in range(2):
                    nc.tensor.matmul(
                        zt1_ps[:, n * 512:(n + 1) * 512], s1_sb[:, k, :],
                        at1_sb[:, k, n * 512:(n + 1) * 512],
                        start=(k == 0), stop=(k == KT - 1))

            # PSUM [f, i]-halves -> NP z [i, f] via PE transposes
            zt1 = wide("zt1")
            z1 = wide("z1")
            for h in range(H):
                CPE[h](halfslice(zt1, h), zt1_ps[:, h * 512:(h + 1) * 512])
                tpb = ps_tr.tile([128, 512], f32, name=f"tpz{h}",
                                 tag=f"tpz{h}", bufs=1)
                for t in range(HT):
                    nc.tensor.transpose(
                        tileslice(tpb, t), tileslice(zt1, h * HT + t),
                        ident[:])
                CPE[h](halfslice(z1, h), tpb[:])

            # keep the PE HAM window busy through stage A (junk transposes
            # gated on z1 so they pace with the chain, not ahead of it)
            dumA = ps_acc.tile([128, 1024], f32, name="acc", tag="acc",
                               bufs=2)
            for _ in range(6):
                nc.tensor.transpose(dumA[:, 0:128], tileslice(z1, 0),
                                    ident[:])

            # ------------- layer-1: r1 = relu(min(1,A/n3)*dinv*z1) ----------
            # lam1 = min(ATH/(dinv*nz), 1)*dinv == min(ATH/nz, dinv)
            nzsq1, nz1, q1, lam1 = sc("nzsq1"), sc("nz1"), sc("q1"), sc("lam1")
            sq_norms(z1, nzsq1, "z1")
            act(nz1, nzsq1, Act.Sqrt)
            vmax(nz1, MIN_NORM)
            recip(q1, nz1)
            nc.vector.tensor_scalar(lam1.full, q1.full, ATH, None, Alu.mult)
            nc.vector.tensor_tensor(lam1.full, lam1.full, dinv_sb[:, 0:NT],
                                    Alu.min)
            r1 = wide("r1")
            for t in range(NDV):
                nc.vector.tensor_scalar(
                    tileslice(r1, t), tileslice(z1, t),
                    lam1.t[:, t:t + 1], 0.0, Alu.mult, Alu.max)
            for t in range(NDV, NT):
                nc.scalar.activation(tileslice(r1, t), tileslice(z1, t),
                                     Act.Relu, scale=lam1.t[:, t:t + 1])
            nrsq1, nr1, qr1 = sc("nrsq1"), sc("nr1"), sc("qr1")
            sq_norms(r1, nrsq1, "r1")
            act(nr1, nrsq1, Act.Sqrt)
            vmax(nr1, MIN_NORM)
            recip(qr1, nr1)              # qr1 = 1/nr1

            # ---------------- layer-2 HypLinear on r1 ----------------------
            r1t = wide("r1t", bf16)
            for h in range(H):
                tpb2 = ps_tr.tile([128, 512], f32, name=f"tpz{h}",
                                  tag=f"tpz{h}", bufs=1)
                for t in range(HT):
                    nc.tensor.transpose(tileslice(tpb2, t),
                                        tileslice(r1, h * HT + t), ident[:])
                CPE[h](halfslice(r1t, h), tpb2[:])
            rxt = wide("rxt")
            for h in range(H):
                mx_ps = ps_acc.tile([128, 512], f32, name="mx_ps", tag="mx_ps",
                                    bufs=2)
                nc.tensor.matmul(mx_ps[:], w2t_sb[:], halfslice(r1t, h),
                                 start=True, stop=True)
                CPE[h](halfslice(rxt, h), mx_ps[:])
            rx = wide("rx")
            for h in range(H):
                tpb3 = ps_tr.tile([128, 512], f32, name=f"tpz{h}",
                                  tag=f"tpz{h}", bufs=1)
                for t in range(HT):
                    nc.tensor.transpose(tileslice(tpb3, t),
                                        tileslice(rxt, h * HT + t), ident[:])
                CPE[h](halfslice(rx, h), tpb3[:])

            # keep the PE warm through the mobius-scalar + s2b stretch
            dumB = ps_acc.tile([128, 1024], f32, name="acc", tag="acc",
                               bufs=2)
            for _ in range(14):
                nc.tensor.transpose(dumB[:, 0:128], tileslice(rx, 0),
                                    ident[:])

            rxnsq, rrxn, rxn = sc("rxnsq"), sc("rrxn"), sc("rxn")
            sq_norms(rx, rxnsq, "rx")
            act(rxn, rxnsq, Act.Sqrt)
            vmax(rxn, MIN_NORM)
            recip(rrxn, rxn)             # rrxn = 1/rxn
            # preload the Tanh table while DVE computes arg + dxr
            nc.scalar.activation(dumm[:], y2[:, 0:1], Act.Tanh)
            clam, arg, tv = sc("clam"), sc("arg"), sc("tv")
            nc.vector.tensor_scalar(clam.full, qr1.full, ATH, 1.0,
                                    Alu.mult, Alu.min)
            nc.vector.tensor_mul(arg.full, clam.full, rxn.full)
            act(tv, arg, Act.Tanh)
            nc.vector.tensor_scalar_min(tv.full, tv.full, MAXN)
            # preload the Sqrt table while DVE computes m/xy/mobius scalars
            nc.scalar.activation(dumm[:], y2[:, 0:1], Act.Sqrt)
            # dot_rx[i] = <rx[i], hb> computed on rx directly -- runs
            # concurrently with the tanh; xy = fm * dot_rx afterwards
            dxr = sc("dxr")
            for h in range(H):
                for t in range(HT):
                    xscr = work.tile([128, 128], f32, name="xscr",
                                     tag=f"nscr{h}", bufs=4)
                    nc.vector.scalar_tensor_tensor(
                        xscr[:], tileslice(rx, h * HT + t), 1.0, hb2_sb[:],
                        Alu.bypass, Alu.mult, accum_out=dxr[h][:, t:t + 1])
            fm = sc("fm")
            nc.vector.tensor_mul(fm.full, tv.full, rrxn.full)
            xy = sc("xy")
            nc.vector.tensor_mul(xy.full, fm.full, dxr.full)
            x2, d1, den, rden = sc("x2"), sc("d1"), sc("den"), sc("rden")
            a0, alpha, b0, beta = sc("a0"), sc("alpha"), sc("b0"), sc("beta")
            nc.vector.tensor_mul(x2.full, tv.full, tv.full)
            nc.vector.tensor_scalar(d1.full, x2.full, y2[:, 0:1], None,
                                    Alu.mult)
            nc.vector.scalar_tensor_tensor(den.full, xy.full, 2.0,
                                           d1.full, Alu.mult, Alu.add)
            nc.vector.tensor_scalar(den.full, den.full, 1.0, MIN_NORM,
                                    Alu.add, Alu.max)
            nc.vector.reciprocal(rden.full, den.full)
            nc.vector.tensor_scalar(a0.full, xy.full, 2.0, 1.0,
                                    Alu.mult, Alu.add)
            nc.vector.tensor_scalar(a0.full, a0.full, y2[:, 0:1], None,
                                    Alu.add)
            nc.vector.tensor_mul(alpha.full, a0.full, rden.full)
            nc.vector.tensor_scalar(b0.full, x2.full, -1.0, 1.0,
                                    Alu.mult, Alu.add)
            nc.vector.tensor_mul(beta.full, b0.full, rden.full)
            # ||m2||^2 analytically: m2 = a*m + b*hb with ||m||=tv,
            # <m,hb>=xy, ||hb||^2=y2  =>  n2^2 = a^2 x2 + 2ab xy + b^2 y2
            n2sq, n2, mn2 = sc("n2sq"), sc("n2"), sc("mn2")
            aa, ab, bb = sc("aa"), sc("ab"), sc("bb")
            nc.vector.tensor_mul(aa.full, alpha.full, alpha.full)
            nc.vector.tensor_mul(ab.full, alpha.full, beta.full)
            nc.vector.tensor_mul(bb.full, beta.full, beta.full)
            nc.vector.tensor_mul(aa.full, aa.full, x2.full)
            nc.vector.scalar_tensor_tensor(ab.full, xy.full, 2.0, ab.full,
                                           Alu.mult, Alu.mult)
            nc.vector.tensor_scalar(bb.full, bb.full, y2[:, 0:1], None,
                                    Alu.mult)
            nc.vector.tensor_add(n2sq.full, aa.full, ab.full)
            nc.vector.tensor_add(n2sq.full, n2sq.full, bb.full)
            rn2 = sc("rn2")
            act(n2, n2sq, Act.Sqrt)
            vmax(n2, MIN_NORM)
            # preload the Ln table while DVE computes the artanh ratio
            nc.scalar.activation(dumm[:], y2[:, 0:1], Act.Ln)
            recip(rn2, n2)
            am1, ram, rat, ln2 = sc("am1"), sc("ram"), sc("rat"), sc("ln2")
            nc.vector.tensor_scalar_min(mn2.full, n2.full, MAXN)
            nc.vector.tensor_scalar(am1.full, mn2.full, -1.0, 1.0,
                                    Alu.mult, Alu.add)
            nc.vector.reciprocal(ram.full, am1.full)
            nc.vector.tensor_scalar(rat.full, ram.full, 2.0, -1.0,
                                    Alu.mult, Alu.add)
            act(ln2, rat, Act.Ln)
            g2, gs = sc("g2"), sc("gs")
            nc.vector.scalar_tensor_tensor(g2.full, ln2.full, 0.5,
                                           rn2.full, Alu.mult, Alu.mult)
            nc.vector.tensor_mul(gs.full, g2.full, dinv_sb[:, NT:2 * NT])
            # s2 = gs*m2 = (gs*a)*m + (gs*b)*hb  (m2 never materialized)
            ga, gb = sc("ga"), sc("gb")
            nc.vector.tensor_mul(ga.full, gs.full, alpha.full)
            nc.vector.tensor_mul(ga.full, ga.full, fm.full)   # fold fm
            nc.vector.tensor_mul(gb.full, gs.full, beta.full)
            s2b = wide("s2b", f8)
            for g in range(NT):
                h = g // HT
                gbhb = work.tile([128, 128], f32, name="gbhb",
                                 tag=f"gbhb{h}", bufs=2)
                if h == 0:
                    nc.vector.tensor_scalar(gbhb[:], hb2_sb[:],
                                            gb.t[:, g:g + 1], None, Alu.mult)
                else:
                    nc.scalar.activation(gbhb[:], hb2_sb[:], Act.Copy,
                                         scale=gb.t[:, g:g + 1])
                nc.vector.scalar_tensor_tensor(
                    tileslice(s2b, g), tileslice(rx, g), ga.t[:, g:g + 1],
                    gbhb[:], Alu.mult, Alu.add)

            # ---------------- GEMM2 (column-parallel, partial z2) ----------
            # fp8 DoubleRow: each matmul contracts a PAIR of j-tiles
            # (lhsT [128,2,128] = two adjacent s2b tiles, rhs [128,2,512])
            s2b3 = s2b[:].rearrange("p (t f) -> p t f", t=NT)
            for c in range(IC):
                zt2_ps = ps_acc.tile([128, 1024], f32, name="acc", tag="acc",
                                     bufs=2)
                for tt in range(NT // 2):
                    for n in range(2):
                        nc.tensor.matmul(
                            zt2_ps[:, n * 512:(n + 1) * 512],
                            s2b3[:, 2 * tt:2 * tt + 2, :],
                            at2_sb[:, c, 2 * tt:2 * tt + 2,
                                   n * 512:(n + 1) * 512],
                            start=(tt == 0), stop=(tt == NT // 2 - 1),
                            perf_mode=mybir.MatmulPerfMode.DoubleRow)
                zt2c = work.tile([128, 1024], bf16, name="zt2c",
                                 tag=f"zt2c{c % 2}", bufs=1)
                CPE[c % 2](zt2c[:], zt2_ps[:])  # alternates DVE / GpSimd
                nc.sync.dma_start(out_ext[:, c * 1024:(c + 1) * 1024],
                                  zt2c[:])

    nc.compile()
    return nc


# ----------------------------------------------------------------------------
# host-side math (fp32, mirrors the device numerics)
# ----------------------------------------------------------------------------

def _np_artanh(x):
    x = np.clip(x, -1.0 + 1e-6, 1.0 - 1e-6)
    return 0.5 * (np.log1p(x) - np.log1p(-x))


def _host_hb(b):
    nb = max(np.linalg.norm(b), MIN_NORM)
    hbn = min(np.tanh(nb), MAXN)
    return ((hbn / nb) * b).astype(np.float32)


def _host_s1(x, dinv, W1, b1):
    x = x.astype(np.float32)
    n = np.maximum(np.linalg.norm(x, axis=-1, keepdims=True), MIN_NORM)
    hn = np.minimum(np.tanh(n), MAXN)
    h = (hn / n) * x
    hb = _host_hb(b1)
    y2 = float(hb @ hb)
    mx = h @ W1.T
    mxn = np.maximum(np.linalg.norm(mx, axis=-1, keepdims=True), MIN_NORM)
    arg = mxn / hn * _np_artanh(hn)
    tv = np.tanh(arg)
    m = (np.minimum(tv, MAXN) / mxn) * mx
    mn = np.minimum(tv, MAXN)
    xy = (m * hb).sum(-1, keepdims=True)
    x2 = mn * mn
    den = np.maximum(1.0 + 2.0 * xy + x2 * y2, MIN_NORM)
    alpha = (1.0 + 2.0 * xy + y2) / den
    beta = (1.0 - x2) / den
    m2 = alpha * m + beta * hb
    n2 = np.maximum(np.linalg.norm(m2, axis=-1, keepdims=True), MIN_NORM)
    g = _np_artanh(np.minimum(n2, MAXN)) / n2
    return (dinv[:, None] * g) * m2


def _erf(x):
    try:
        from scipy.special import erf
        return erf(x)
    except Exception:
        return np.vectorize(math.erf)(x).astype(x.dtype)


def _pmajor(a, nt):
    """[nt*128, w] row-major -> [128, nt*w] partition-major."""
    w = a.shape[1]
    return np.ascontiguousarray(
        a.reshape(nt, 128, w).transpose(1, 0, 2).reshape(128, nt * w))


_NC_CACHE = {}


def kernel(x1, x2, adj1, adj2, dist1, dist2, W_stack, b_stack,
           W_mlp1, b_mlp1, W_mlp2, b_mlp2):
    x1 = np.asarray(x1, np.float32)
    x2 = np.asarray(x2, np.float32)
    W_stack = np.asarray(W_stack, np.float32)
    b_stack = np.asarray(b_stack, np.float32)

    if "nc" not in _NC_CACHE:
        _NC_CACHE["nc"] = build_nc()
    nc = _NC_CACHE["nc"]

    # ---- host prep per graph ----
    in_maps = [None] * N_CORES
    dinv_full = [None, None]
    for g, adj, x in ((0, adj1, x1), (1, adj2, x2)):
        a = np.asarray(adj, np.float32)[0]
        at = np.ascontiguousarray(a.T)
        idx = np.arange(N)
        at[idx, idx] += LOOP_ATT
        at_bf = at.astype(ml_dtypes.bfloat16)
        at_q = at.astype(ml_dtypes.float8_e4m3)
        deg = at_bf.astype(np.float32).sum(axis=0)
        dinv = deg.astype(np.float32) ** -0.5
        dinv_full[g] = dinv
        s1 = _host_s1(x[0], dinv, W_stack[0], b_stack[0])
        s1_pm = _pmajor(s1.astype(ml_dtypes.bfloat16), KT)
        w2t = np.ascontiguousarray(W_stack[1].T).astype(ml_dtypes.bfloat16)
        hb2 = np.broadcast_to(_host_hb(b_stack[1]), (128, F)).copy()
        for c in range(CPG):
            core = g * CPG + c
            r0 = c * RL
            at1_pm = _pmajor(at_bf[:, r0:r0 + RL], KT)
            # at2: local j rows x all i, fp8, [128, (c,t,i)]-major
            at2_pm = np.ascontiguousarray(
                at_q[r0:r0 + RL, :].reshape(NT, 128, IC, 1024)
                .transpose(1, 2, 0, 3).reshape(128, IC * NT * 1024))
            dloc = dinv[r0:r0 + RL].reshape(NT, 128).T.astype(np.float32)
            dinv_np = np.ascontiguousarray(np.hstack([dloc, dloc * S2]))
            in_maps[core] = {
                "at1": at1_pm,
                "at2": at2_pm,
                "s1": s1_pm,
                "dinv": dinv_np,
                "w2t": w2t,
                "hb2": hb2,
            }

    res = bass_utils.run_bass_kernel_spmd(nc, in_maps,
                                          core_ids=list(range(N_CORES)))

    # ---- host epilogue: partial-z2 sum + layer-2 HypAct + readout + MLP ----
    outs = []
    for g in range(2):
        zt2 = np.zeros((128, N), np.float32)
        for c in range(CPG):
            zt2 += res.results[g * CPG + c]["out"].astype(np.float32)
        z2 = zt2.T / S2  # [N, F], undo the fp8 s2 scale
        nz = np.linalg.norm(z2, axis=-1, keepdims=True)
        n3 = np.maximum(dinv_full[g][:, None] * nz, MIN_NORM)
        lam = np.minimum(1.0, ATH / n3) * dinv_full[g][:, None]
        r2 = np.maximum(lam * z2, 0.0)
        nr2 = np.maximum(np.linalg.norm(r2, axis=-1, keepdims=True), MIN_NORM)
        h = (np.minimum(np.tanh(nr2), MAXN) / nr2) * r2
        outs.append(np.concatenate([h.mean(axis=0), h.max(axis=0)]))
    feat = np.concatenate([outs[0], outs[1], outs[0] - outs[1]]).astype(np.float32)
    zmlp = feat @ np.asarray(W_mlp1, np.float32).T + np.asarray(b_mlp1, np.float32)
    hidden = 0.5 * zmlp * (1.0 + _erf(zmlp / np.sqrt(2.0).astype(np.float32)))
    logits = hidden @ np.asarray(W_mlp2, np.float32).T + np.asarray(
        b_mlp2, np.float32)
    return logits.astype(np.float32)
